# revision 47
# baseline (speedup 1.0000x reference)
"""Causal attention block (LN -> QKV -> causal MHA -> out-proj) on 8 trn2
NeuronCores via Bass/Tile.

Sharding: core c handles batch b=c//2 and head-group g=c%2 (8 of 16 heads).
Data parallel over batch, tensor parallel over heads; the out-proj partial
sums (2 per batch) are reduced on the host during the gather, so the device
program needs no collectives and is pure SPMD.

v2 design (bf16 everywhere, PE kept warm, no DRAM roundtrips):
  - all matmul operands bf16 (full-rate 1 cyc/row, FWL weight loads, half
    the DMA); PSUM accumulation stays fp32.
  - LN stats land in a [4, 512] PSUM layout so the postprocessing runs on
    4 partitions instead of 1; rstd is folded into the PSUM->SBUF copies
    (per-partition tensor_scalar for V, a broadcast-row multiply for Q/K),
    so x itself is never rewritten.  The mean/beta correction enters each
    projection as a K=2 seed matmul with rows [std, -mu].
  - scores are computed transposed per 128-key chunk with 2 heads packed;
    exp runs on ACT straight out of PSUM with the 1/sqrt(dh) scale folded
    in; diagonal chunks are narrowed to the valid q-range (saves PE + ACT)
    and only the 128x128 boundary block gets a 0/1 mask multiply.
  - softmax denominators ride as a 65th ones-column of V through the P@V
    matmul; per pair they are DMA-gathered into a [64, 64] tile for one
    batched reciprocal, spread back with two small DMAs, and applied in the
    normalize copy into the SBUF-resident O^T (no DRAM scratch).
  - the QK projection of pair p+1 is interleaved into the (ACT-bound)
    attention stream of pair p so the PE never idles long enough to be
    clock-throttled.
"""

import numpy as np

import concourse.bass as bass
import concourse.mybir as mybir
import concourse.tile as tile_mod

# ----------------------------------------------------------------------------
# Workaround for this walrus build rejecting instructions that carry more than
# MAX_WAITS semaphore waits ("Too many sync wait commands" in CoreV3GenImpl
# setupSyncWait — hit on Drain and Matmult/S3_LW encodings). Split excess
# waits onto single-wait NOP carrier instructions emitted just before the
# original instruction on the same engine: program order on the sequencer
# makes this semantically identical (waits are AND conditions).
# ----------------------------------------------------------------------------
_MAX_WAITS = 1
_orig_add_instruction = tile_mod.TileContext._add_instruction
_carrier_id = [0]


def _split_waits_add_instruction(self, inst):
    si = inst.sync_info
    if (
        si is not None
        and si.on_wait
        and len(si.on_wait) > _MAX_WAITS
        and inst.engine != mybir.EngineType.Unassigned
    ):
        waits = list(si.on_wait)
        keep = waits[-_MAX_WAITS:]
        for w in waits[:-_MAX_WAITS]:
            _carrier_id[0] += 1
            nop = mybir.InstNoOp(name=f"I-waitc-{_carrier_id[0]}")
            nop.engine = inst.engine
            nop.sync_info = mybir.SyncInfo(on_wait=[w], on_update=[])
            _orig_add_instruction(self, nop)
        inst.sync_info = mybir.SyncInfo(
            on_wait=keep,
            on_update=list(si.on_update) if si.on_update else [],
        )
    _orig_add_instruction(self, inst)


tile_mod.TileContext._add_instruction = _split_waits_add_instruction

from concourse.vector_clock import ScopedClock


def _patched_drain_and_barrier(self, tick_clock, wait_clock):
    # Same wait-splitting for the TileContext exit drain, which is emitted
    # after lowering (outside _add_instruction).
    nc = self.nc
    carrier = nc.sync.nop(nofuse=True)
    wait_clock.add_sem_waits(carrier.ins, ScopedClock({None: tick_clock.global_clock}))
    si = carrier.ins.sync_info
    waits = list(si.on_wait) if si is not None and si.on_wait else []
    if len(waits) > _MAX_WAITS:
        carrier.ins.sync_info = mybir.SyncInfo(
            on_wait=waits[:_MAX_WAITS],
            on_update=list(si.on_update) if si.on_update else [],
        )
        rest = waits[_MAX_WAITS:]
        while rest:
            extra = nc.sync.nop(nofuse=True)
            extra.ins.sync_info = mybir.SyncInfo(
                on_wait=rest[:_MAX_WAITS], on_update=[])
            rest = rest[_MAX_WAITS:]

    nc.sync.drain()
    nc.all_engine_barrier()
    assert self.sems is not None
    popped = nc._tile_sem_poison_stack.pop()
    assert popped is self._sem_poison
    nc.clear_and_free_semaphores(list(self.sems.allocated().values()))
    nc.all_engine_barrier()


tile_mod.TileContext._drain_and_barrier = _patched_drain_and_barrier

# ----------------------------------------------------------------------------

F32 = mybir.dt.float32
BF16 = mybir.dt.bfloat16
ALU = mybir.AluOpType
ACT_EXP = mybir.ActivationFunctionType.Exp
ACT_SQRT = mybir.ActivationFunctionType.Sqrt

B = 4
TOK = 2048
DIM = 1024
HEADS = 16
DH = 64
HEADS_PC = 8          # heads per core
INNER_PC = HEADS_PC * DH  # 512
NPAIRS = HEADS_PC // 2
QT = 512              # query tile
KC = 128              # key-token chunk
NQT = TOK // QT       # 4
NTT = TOK // KC       # 16
NKD = DIM // 128      # 8
EPS = 1e-5
SCALE = DH ** -0.5


def _ap0(ap, parts):
    """Partition-broadcast AP (stride-0 leading dim) for DMA reads."""
    return bass.AP(tensor=ap.tensor, offset=ap.offset, ap=[[0, parts]] + list(ap.ap))


def _pstride(t, step, n):
    """View tile t's partitions with a stride (partition p -> p*step)."""
    return bass.AP(tensor=t.tensor, offset=t.offset,
                   ap=[[t.ap[0][0] * step, n]] + list(t.ap[1:]))


def build_program(tok=TOK):
    nc = bass.Bass()
    xT = nc.declare_dram_parameter("xT", [DIM, tok], BF16, isOutput=False)
    w = nc.declare_dram_parameter("w", [DIM, 3 * INNER_PC], BF16, isOutput=False)
    seed = nc.declare_dram_parameter("seed", [2, 3 * INNER_PC], BF16, isOutput=False)
    wo = nc.declare_dram_parameter("wo", [INNER_PC, DIM], BF16, isOutput=False)
    mask_d = nc.declare_dram_parameter("mask128", [KC, KC], BF16, isOutput=False)
    eye_d = nc.declare_dram_parameter("eye4", [4, 4], F32, isOutput=False)
    out = nc.declare_dram_parameter("out", [tok, DIM], F32, isOutput=True)
    # DRAM bounce buffers for partition-broadcasts (stride-0 reads are only
    # legal on the DRAM side of a DMA)
    rstd_d = nc.dram_tensor("rstd_row", [1, tok], F32)
    rec_d = nc.dram_tensor("rec_row", [NPAIRS, 8 * QT], BF16)

    with tile_mod.TileContext(nc) as tc, nc.allow_low_precision(
            "bf16 operand tiles; all matmul accumulation stays fp32 PSUM"):
        with (
            tc.tile_pool(name="const", bufs=1) as const,
            tc.tile_pool(name="xt", bufs=NKD) as xtp,
            tc.tile_pool(name="wsb", bufs=NKD) as wp,
            tc.tile_pool(name="wosb", bufs=NPAIRS) as wop,
            tc.tile_pool(name="vpool", bufs=NTT) as vpool,
            tc.tile_pool(name="qkT", bufs=2) as qkp,
            tc.tile_pool(name="osb", bufs=3) as osbp,
            tc.tile_pool(name="oT", bufs=NPAIRS) as oTp,
            tc.tile_pool(name="psb", bufs=3) as ppool,
            tc.tile_pool(name="den", bufs=2) as denp,
            tc.tile_pool(name="sq", bufs=2) as sqp,
            tc.tile_pool(name="out_sb", bufs=4) as outp,
        ):
            # ---------------- constants / big loads ----------------
            ones_col = const.tile([128, 1], BF16, tag="ones_col")
            nc.vector.memset(ones_col, 1.0)
            # sel4: 4 stationary variants [128, 4]; variant v is all-ones in
            # column v, zero elsewhere.  Routes LN stat rows to consecutive
            # PSUM partitions (matmul/DVE PSUM bases must be 32-aligned).
            sel4 = const.tile([128, 16], BF16, tag="sel4")
            nc.vector.memset(sel4, 0.0)
            for v in range(4):
                nc.vector.memset(sel4[:, 5 * v:5 * v + 1], 1.0)
            eps4 = const.tile([4, 1], F32, tag="eps")
            nc.vector.memset(eps4, EPS)
            mask_sb = const.tile([KC, KC], BF16, tag="mask")
            nc.sync.dma_start(out=mask_sb, in_=mask_d[:, :])
            eye4 = const.tile([4, 4], F32, tag="eye4")
            nc.sync.dma_start(out=eye4, in_=eye_d[:, :])
            seed_sb = const.tile([2, 3 * INNER_PC], BF16, tag="seed")
            nc.sync.dma_start(out=seed_sb, in_=seed[:, :])

            # x chunks arrive split in token-halves across four engine
            # queues so the transfers run in parallel and the LN stats start
            # consuming as early as possible.
            xt = []
            qeng = [nc.sync, nc.scalar, nc.gpsimd]
            for kc in range(NKD):
                t = xtp.tile([128, tok], BF16, tag="xt")
                for hf in range(2):
                    hsl = slice(hf * (tok // 2), (hf + 1) * (tok // 2))
                    qeng[(2 * kc + hf) % 3].dma_start(
                        out=t[:, hsl],
                        in_=xT[kc * 128:(kc + 1) * 128, hsl])
                xt.append(t)
            wsb = []
            for kc in range(NKD):
                t = wp.tile([128, 3 * INNER_PC], BF16, tag="wsb")
                nc.sync.dma_start(out=t, in_=w[kc * 128:(kc + 1) * 128, :])
                wsb.append(t)
            wos = []
            for p in range(NPAIRS):
                t = wop.tile([128, DIM], BF16, tag="wosb")
                nc.sync.dma_start(out=t, in_=wo[p * 128:(p + 1) * 128, :])
                wos.append(t)

            # LN-derived rows (device computed, long-lived)
            onm = const.tile([2, tok], BF16, tag="onm")        # r0=std r1=-mu
            bc_sb = const.tile([128, tok], F32, tag="bc")      # rstd bcast
            # NOTE: rstd_col columns are block-permuted: chunk tt lives at
            # column 4*(tt%4) + tt//4 (transpose blocks land contiguously).
            rstd_col = const.tile([128, NTT], F32, tag="rstdc")
            rstd4 = const.tile([4, QT], F32, tag="rstd4")
            std4 = const.tile([4, QT], F32, tag="std4")
            std4b = const.tile([4, QT], BF16, tag="std4b")
            nmu4b = const.tile([4, QT], BF16, tag="nmu4b")
            mu4 = const.tile([4, QT], F32, tag="mu4")
            var4 = const.tile([4, QT], F32, tag="var4")
            musq4 = const.tile([4, QT], F32, tag="musq4")

            # ---------------- phase A: LN stats ----------------
            with (
                tc.tile_pool(name="ps_stats", bufs=1, space="PSUM") as pstat,
            ):
                # Warm-up: ~4us of dummy matmuls on constants while the x
                # DMAs land, so the HAM clock gate releases (1.2 -> 2.4 GHz)
                # before the real work starts.
                warm_ps = pstat.tile([4, KC], F32, tag="warm")
                for _ in range(40):
                    nc.tensor.matmul(out=warm_ps, lhsT=sel4[:, 0:4],
                                     rhs=mask_sb, start=True, stop=True)
                # Two [4, 512] PSUM tiles: token-slice nt's sum / sum-of-sq
                # rows land on partition nt via the sel4 stationary (other
                # rows accumulate zero), one accumulation group per tile.
                sum_ps = pstat.tile([4, QT], F32, tag="sum")
                sq_ps = pstat.tile([4, QT], F32, tag="sq")
                for kc in range(NKD):
                    for nt in range(NQT):
                        sl = slice(nt * QT, (nt + 1) * QT)
                        first = kc == 0 and nt == 0
                        last = kc == NKD - 1 and nt == NQT - 1
                        sq_t = sqp.tile([128, QT], BF16, tag="sq_t")
                        nc.vector.tensor_mul(sq_t, xt[kc][:, sl], xt[kc][:, sl])
                        nc.tensor.matmul(
                            out=sum_ps, lhsT=sel4[:, 4 * nt:4 * nt + 4],
                            rhs=xt[kc][:, sl],
                            start=first, stop=last)
                        nc.tensor.matmul(
                            out=sq_ps, lhsT=sel4[:, 4 * nt:4 * nt + 4],
                            rhs=sq_t,
                            start=first, stop=last)
                # postproc on [4, 512] (4 lanes)
                nc.vector.tensor_scalar_mul(mu4, sum_ps, 1.0 / DIM)
                nc.vector.tensor_scalar_mul(var4, sq_ps, 1.0 / DIM)
                nc.vector.tensor_mul(musq4, mu4, mu4)
                nc.vector.tensor_sub(var4, var4, musq4)
                nc.scalar.activation(out=std4, in_=var4, func=ACT_SQRT,
                                     bias=eps4, scale=1.0)
                nc.vector.reciprocal(rstd4, std4)
                nc.vector.tensor_copy(std4b, std4)
                nc.vector.tensor_scalar_mul(nmu4b, mu4, -1.0)
                # gather LN rows into operand layouts (cross-shape DMAs:
                # only total element count must match)
                # rstd broadcast [128, tok]: bounce through DRAM, then
                # stride-0 partition-broadcast reads (split per 512-slice so
                # the first Q/K copies unblock as early as possible)
                nc.gpsimd.dma_start(out=rstd_d[0:1, :], in_=rstd4[:, :])
                nc.gpsimd.dma_start(out=onm[0:1, :], in_=std4b[:, :])
                nc.gpsimd.dma_start(out=onm[1:2, :], in_=nmu4b[:, :])
                for j in range(NQT):
                    nc.gpsimd.dma_start(
                        out=bc_sb[:, j * QT:(j + 1) * QT],
                        in_=bass.AP(tensor=rstd_d, offset=j * QT,
                                    ap=[[0, 128], [1, QT]]))
            # ------- phases B-D: projections + attention + out-proj -------
            # One shared [128, 512]-f32 PSUM pool ("proj") serves the QK
            # slices, the V groups, the rstd transposes and the out-proj
            # groups; they never overlap in time.  8 banks total:
            # proj 2 + scores 4 + o_ps 2.
            v_sb = [None] * NTT
            oTs = []
            with (
                tc.tile_pool(name="ps_proj", bufs=2, space="PSUM") as pproj,
                tc.tile_pool(name="ps_s", bufs=2, space="PSUM") as pss,
                tc.tile_pool(name="ps_o", bufs=2, space="PSUM") as pso,
            ):
                def emit_qk_slice(p, dst, d, nt):
                    """One [128, 512] token-slice of the Q or K projection of
                    pair p (d=0 -> Q, d=1 -> K).  Seed matmul last so the
                    group never waits on the LN postprocessing."""
                    sl = slice(nt * QT, (nt + 1) * QT)
                    cofs = d * INNER_PC + p * 128
                    ps = pproj.tile([128, QT], F32, tag="proj", name="qk_ps")
                    for kc in range(NKD):
                        nc.tensor.matmul(
                            out=ps, lhsT=wsb[kc][:, cofs:cofs + 128],
                            rhs=xt[kc][:, sl],
                            start=(kc == 0), stop=False)
                    nc.tensor.matmul(
                        out=ps, lhsT=seed_sb[:, cofs:cofs + 128],
                        rhs=onm[:, sl], start=False, stop=True)
                    nc.vector.tensor_mul(dst[:, sl], ps, bc_sb[:, sl])

                def emit_rstd_col():
                    # 4 PE transposes of [4, 128] blocks; block g lands at
                    # contiguous cols [4g, 4g+4) (permuted order, see above).
                    rc_ps = pproj.tile([128, QT], F32, tag="proj", name="rc")
                    for g in range(4):
                        nc.tensor.transpose(
                            out=rc_ps[:, 4 * g:4 * (g + 1)],
                            in_=rstd4[:, g * 128:(g + 1) * 128],
                            identity=eye4)
                    nc.vector.tensor_copy(rstd_col, rc_ps[:, 0:NTT])

                def emit_v_group(tt):
                    tsl = slice(tt * KC, (tt + 1) * KC)
                    v_ps = pproj.tile([128, INNER_PC], F32, tag="proj",
                                      name="v_ps")
                    for kc in range(NKD):
                        nc.tensor.matmul(
                            out=v_ps, lhsT=xt[kc][:, tsl],
                            rhs=wsb[kc][:, 2 * INNER_PC:3 * INNER_PC],
                            start=(kc == 0), stop=False)
                    nc.tensor.matmul(
                        out=v_ps, lhsT=onm[:, tsl],
                        rhs=seed_sb[:, 2 * INNER_PC:3 * INNER_PC],
                        start=False, stop=True)
                    vt = vpool.tile([128, HEADS_PC * (DH + 1)], BF16,
                                    tag="v_sb", name=f"v_sb{tt}")
                    v3 = vt.rearrange("p (h w) -> p h w", w=DH + 1)
                    pc = 4 * (tt % 4) + tt // 4  # permuted rstd_col index
                    nc.vector.tensor_scalar(
                        out=v3[:, :, 0:DH],
                        in0=v_ps.rearrange("p (h w) -> p h w", w=DH),
                        scalar1=rstd_col[:, pc:pc + 1], scalar2=None,
                        op0=ALU.mult)
                    nc.vector.memset(v3[:, :, DH:DH + 1], 1.0)
                    v_sb[tt] = vt

                def emit_outproj_tt(tt):
                    tsl = slice(tt * KC, (tt + 1) * KC)
                    for nb in range(DIM // QT):
                        nsl = slice(nb * QT, (nb + 1) * QT)
                        ps = pproj.tile([128, QT], F32, tag="proj",
                                        name="out_ps")
                        for p in range(NPAIRS):
                            nc.tensor.matmul(
                                out=ps, lhsT=oTs[p][:, tsl],
                                rhs=wos[p][:, nsl],
                                start=(p == 0), stop=(p == NPAIRS - 1))
                        ob = outp.tile([128, QT], F32, tag="out_sb")
                        nc.vector.tensor_copy(ob, ps)
                        nc.sync.dma_start(out=out[tsl, nsl], in_=ob)

                def emit_attn_qtile(p, t_i, qT, kT, oT):
                    """Scores/exp/mask/PV for one query tile, then the
                    per-qtile denominator chain and normalize into oT."""
                    qsl0 = t_i * QT
                    nch = (t_i + 1) * QT // KC
                    o_ps = [pso.tile([DH + 1, QT], F32, tag="o_ps",
                                     name=f"o_ps{p}_{t_i}_{h}")
                            for h in range(2)]
                    p_tiles = {}

                    def emit_scores(c):
                        m = c - (nch - 4)
                        lo = 128 * m if m > 0 else 0
                        csl = slice(c * KC, (c + 1) * KC)
                        s_ps = pss.tile([128, 2 * QT], F32, tag="s_ps")
                        p_sb = ppool.tile([128, 2 * QT], BF16, tag="p_sb")
                        for h in range(2):
                            nc.tensor.matmul(
                                out=s_ps[:, h * QT + lo:(h + 1) * QT],
                                lhsT=kT[h * DH:(h + 1) * DH, csl],
                                rhs=qT[h * DH:(h + 1) * DH,
                                       qsl0 + lo:qsl0 + QT],
                                start=True, stop=True)
                        s3 = s_ps.rearrange("p (h q) -> p h q", q=QT)
                        p3 = p_sb.rearrange("p (h q) -> p h q", q=QT)
                        nc.scalar.activation(
                            out=p3[:, :, lo:QT], in_=s3[:, :, lo:QT],
                            func=ACT_EXP, scale=SCALE)
                        if m >= 0:
                            for h in range(2):
                                nc.vector.tensor_mul(
                                    p_sb[:, h * QT + lo:h * QT + lo + KC],
                                    p_sb[:, h * QT + lo:h * QT + lo + KC],
                                    mask_sb)
                        p_tiles[c] = p_sb

                    def emit_pv(c):
                        m = c - (nch - 4)
                        lo = 128 * m if m > 0 else 0
                        p_sb = p_tiles.pop(c)
                        for h in range(2):
                            hc = (2 * p + h) * (DH + 1)
                            nc.tensor.matmul(
                                out=o_ps[h][:, lo:QT],
                                lhsT=v_sb[c][:, hc:hc + DH + 1],
                                rhs=p_sb[:, h * QT + lo:(h + 1) * QT],
                                start=(c == 0), stop=(c == nch - 1),
                                skip_group_check=True)

                    emit_scores(0)
                    for c in range(1, nch):
                        emit_scores(c)
                        emit_pv(c - 1)
                    emit_pv(nch - 1)
                    # free PSUM fast (O^T rows + denominator row 64)
                    o_sb = [osbp.tile([DH + 1, QT], BF16, tag=f"o_sb{h}",
                                      name=f"o_sb{p}_{t_i}_{h}")
                            for h in range(2)]
                    for h in range(2):
                        nc.vector.tensor_copy(o_sb[h], o_ps[h])
                    # per-qtile denominator chain: den16 partition 8h+j
                    # holds tokens [64j, 64j+64) of head h; flattened
                    # partition-major this gives rec_d offsets 512h+64j+e.
                    den16 = denp.tile([16, 64], BF16, tag="den16")
                    rec16 = denp.tile([16, 64], BF16, tag="rec16")
                    rb_q = denp.tile([64, 2 * QT], BF16, tag="rb_q")
                    for h in range(2):
                        nc.gpsimd.dma_start(
                            out=den16[8 * h:8 * h + 8, :],
                            in_=o_sb[h][DH:DH + 1, :])
                    nc.vector.reciprocal(rec16, den16)
                    dofs = p * 8 * QT + t_i * 2 * QT
                    nc.gpsimd.dma_start(
                        out=bass.AP(tensor=rec_d, offset=dofs,
                                    ap=[[2 * QT, 1], [1, 2 * QT]]),
                        in_=rec16[:, :])
                    nc.gpsimd.dma_start(
                        out=rb_q,
                        in_=bass.AP(tensor=rec_d, offset=dofs,
                                    ap=[[0, 64], [1, 2 * QT]]))
                    qsl = slice(qsl0, qsl0 + QT)
                    for h in range(2):
                        nc.vector.tensor_mul(
                            oT[h * DH:(h + 1) * DH, qsl],
                            o_sb[h][0:DH, :],
                            rb_q[:, h * QT:(h + 1) * QT])

                # ---- schedule ----
                # V groups lead: their PSUM-freeing copies gate only on
                # rstd_col (ready right after the postproc), while the q/k
                # copies wait for the slower bc broadcast chain — so V in
                # front keeps the proj pool cycling during the LN latency.
                qT = qkp.tile([128, tok], BF16, tag="qT", name="qT0")
                kT = qkp.tile([128, tok], BF16, tag="kT", name="kT0")
                emit_rstd_col()
                for tt in range(2):
                    emit_v_group(tt)
                emit_qk_slice(0, qT, 0, 0)
                emit_qk_slice(0, kT, 1, 0)
                for tt in range(2, 6):
                    emit_v_group(tt)

                for p in range(NPAIRS):
                    oT = oTp.tile([128, tok], BF16, tag="oT", name=f"oT{p}")
                    oTs.append(oT)
                    nxt = []
                    if p + 1 < NPAIRS:
                        qT2 = qkp.tile([128, tok], BF16, tag="qT",
                                       name=f"qT{p + 1}")
                        kT2 = qkp.tile([128, tok], BF16, tag="kT",
                                       name=f"kT{p + 1}")
                        nxt = [(p + 1, dst, d, nt)
                               for d, dst in ((0, qT2), (1, kT2))
                               for nt in range(NQT)]
                    for t_i in range(NQT):
                        emit_attn_qtile(p, t_i, qT, kT, oT)
                        if p == 0 and t_i < 3:
                            # just-in-time rest of pair 0's QK and V
                            # (V groups 0-5 were emitted up front)
                            emit_qk_slice(0, qT, 0, t_i + 1)
                            emit_qk_slice(0, kT, 1, t_i + 1)
                            for tt in range(4 * t_i + 6, 4 * t_i + 10):
                                if tt < NTT:
                                    emit_v_group(tt)
                        if p == NPAIRS - 1 and t_i < NQT - 1:
                            # out-proj for this qtile's tokens rides along
                            # (the last qtile's blocks run after the pools
                            # close, pairs 0-2 first, so they overlap the
                            # final denominator chain)
                            for tt in range(4 * t_i, 4 * (t_i + 1)):
                                emit_outproj_tt(tt)
                        for _ in range(2):
                            if nxt:
                                emit_qk_slice(*nxt.pop(0))
                    while nxt:
                        emit_qk_slice(*nxt.pop(0))
                    if p + 1 < NPAIRS:
                        qT, kT = qT2, kT2

            # Final out-proj blocks (tokens of the last qtile): pairs 0-2
            # accumulate while pair 3's last denominator chain completes,
            # the pair-3 matmul joins last.
            with tc.tile_pool(name="ps_fin", bufs=6, space="PSUM") as pfin:
                fin = [(tt, nb) for tt in range(4 * (NQT - 1), NTT)
                       for nb in range(DIM // QT)]
                tiles = {}

                def fin_p012(i):
                    tt, nb = fin[i]
                    ps = pfin.tile([128, QT], F32, tag="fin",
                                   name=f"fin{tt}_{nb}")
                    for p in range(NPAIRS - 1):
                        nc.tensor.matmul(
                            out=ps, lhsT=oTs[p][:, tt * KC:(tt + 1) * KC],
                            rhs=wos[p][:, nb * QT:(nb + 1) * QT],
                            start=(p == 0), stop=False)
                    tiles[i] = ps

                for i in range(6):
                    fin_p012(i)
                for i in range(len(fin)):
                    if i >= 6:
                        fin_p012(i)
                    tt, nb = fin[i]
                    nc.tensor.matmul(
                        out=tiles[i],
                        lhsT=oTs[NPAIRS - 1][:, tt * KC:(tt + 1) * KC],
                        rhs=wos[NPAIRS - 1][:, nb * QT:(nb + 1) * QT],
                        start=False, stop=True)
                    ob = outp.tile([128, QT], F32, tag="out_sb")
                    nc.vector.tensor_copy(ob, tiles.pop(i))
                    nc.sync.dma_start(
                        out=out[tt * KC:(tt + 1) * KC,
                                nb * QT:(nb + 1) * QT], in_=ob)

    return nc


def make_masks():
    import ml_dtypes

    k = np.arange(KC)[:, None]
    q = np.arange(KC)[None, :]
    return (q >= k).astype(ml_dtypes.bfloat16)


def make_in_maps(x, ln_gamma, ln_beta, w_qkv, w_out):
    import ml_dtypes

    bf16 = ml_dtypes.bfloat16
    x = np.asarray(x, np.float32)
    g_ = np.asarray(ln_gamma, np.float32)
    b_ = np.asarray(ln_beta, np.float32)
    w_qkv = np.asarray(w_qkv, np.float32)
    w_out = np.asarray(w_out, np.float32)
    mask128 = make_masks()
    eye4 = np.eye(4, dtype=np.float32)
    in_maps = []
    for c in range(8):
        b = c // 2
        g = c % 2
        cs = slice(g * INNER_PC, (g + 1) * INNER_PC)
        Wraw = np.concatenate(
            [w_qkv[:, 0 * DIM:1 * DIM][:, cs],
             w_qkv[:, 1 * DIM:2 * DIM][:, cs],
             w_qkv[:, 2 * DIM:3 * DIM][:, cs]], axis=1)
        Wp = (Wraw * g_[:, None]).astype(bf16)
        seed = np.stack([b_ @ Wraw,
                         Wp.astype(np.float32).sum(axis=0)]).astype(bf16)
        in_maps.append({
            "xT": np.ascontiguousarray(x[b].T).astype(bf16),
            "w": np.ascontiguousarray(Wp),
            "seed": seed,
            "wo": np.ascontiguousarray(w_out[cs, :]).astype(bf16),
            "mask128": mask128,
            "eye4": eye4,
        })
    return in_maps


_PROG = None


def kernel(x, ln_gamma, ln_beta, w_qkv, w_out):
    global _PROG
    from concourse.bass_utils import run_bass_kernel_spmd

    if _PROG is None:
        _PROG = build_program(TOK)
    in_maps = make_in_maps(x, ln_gamma, ln_beta, w_qkv, w_out)
    res = run_bass_kernel_spmd(_PROG, in_maps, list(range(8)))
    parts = [res.results[c]["out"] for c in range(8)]
    out = np.empty((B, TOK, DIM), np.float32)
    for b in range(B):
        out[b] = parts[2 * b] + parts[2 * b + 1]
    return out


# revision 48
# speedup vs baseline: 1.1442x; 1.1442x over previous
"""Causal attention block (LN -> QKV -> causal MHA -> out-proj) on 8 trn2
NeuronCores via Bass/Tile.

Sharding: core c handles batch b=c//2 and head-group g=c%2 (8 of 16 heads).
Data parallel over batch, tensor parallel over heads; the out-proj partial
sums (2 per batch) are reduced on the host during the gather, so the device
program needs no collectives and is pure SPMD.

v2 design (bf16 everywhere, PE kept warm, no DRAM roundtrips):
  - all matmul operands bf16 (full-rate 1 cyc/row, FWL weight loads, half
    the DMA); PSUM accumulation stays fp32.
  - LN stats land in a [4, 512] PSUM layout so the postprocessing runs on
    4 partitions instead of 1; rstd is folded into the PSUM->SBUF copies
    (per-partition tensor_scalar for V, a broadcast-row multiply for Q/K),
    so x itself is never rewritten.  The mean/beta correction enters each
    projection as a K=2 seed matmul with rows [std, -mu].
  - scores are computed transposed per 128-key chunk with 2 heads packed;
    exp runs on ACT straight out of PSUM with the 1/sqrt(dh) scale folded
    in; diagonal chunks are narrowed to the valid q-range (saves PE + ACT)
    and only the 128x128 boundary block gets a 0/1 mask multiply.
  - softmax denominators ride as a 65th ones-column of V through the P@V
    matmul; per pair they are DMA-gathered into a [64, 64] tile for one
    batched reciprocal, spread back with two small DMAs, and applied in the
    normalize copy into the SBUF-resident O^T (no DRAM scratch).
  - the QK projection of pair p+1 is interleaved into the (ACT-bound)
    attention stream of pair p so the PE never idles long enough to be
    clock-throttled.
"""

import numpy as np

import concourse.bass as bass
import concourse.mybir as mybir
import concourse.tile as tile_mod

# ----------------------------------------------------------------------------
# Workaround for this walrus build rejecting instructions that carry more than
# MAX_WAITS semaphore waits ("Too many sync wait commands" in CoreV3GenImpl
# setupSyncWait — hit on Drain and Matmult/S3_LW encodings). Split excess
# waits onto single-wait NOP carrier instructions emitted just before the
# original instruction on the same engine: program order on the sequencer
# makes this semantically identical (waits are AND conditions).
# ----------------------------------------------------------------------------
_MAX_WAITS = 1
_orig_add_instruction = tile_mod.TileContext._add_instruction
_carrier_id = [0]


def _split_waits_add_instruction(self, inst):
    si = inst.sync_info
    if (
        si is not None
        and si.on_wait
        and len(si.on_wait) > _MAX_WAITS
        and inst.engine != mybir.EngineType.Unassigned
    ):
        waits = list(si.on_wait)
        keep = waits[-_MAX_WAITS:]
        for w in waits[:-_MAX_WAITS]:
            _carrier_id[0] += 1
            nop = mybir.InstNoOp(name=f"I-waitc-{_carrier_id[0]}")
            nop.engine = inst.engine
            nop.sync_info = mybir.SyncInfo(on_wait=[w], on_update=[])
            _orig_add_instruction(self, nop)
        inst.sync_info = mybir.SyncInfo(
            on_wait=keep,
            on_update=list(si.on_update) if si.on_update else [],
        )
    _orig_add_instruction(self, inst)


tile_mod.TileContext._add_instruction = _split_waits_add_instruction

from concourse.vector_clock import ScopedClock


def _patched_drain_and_barrier(self, tick_clock, wait_clock):
    # Same wait-splitting for the TileContext exit drain, which is emitted
    # after lowering (outside _add_instruction).
    nc = self.nc
    carrier = nc.sync.nop(nofuse=True)
    wait_clock.add_sem_waits(carrier.ins, ScopedClock({None: tick_clock.global_clock}))
    si = carrier.ins.sync_info
    waits = list(si.on_wait) if si is not None and si.on_wait else []
    if len(waits) > _MAX_WAITS:
        carrier.ins.sync_info = mybir.SyncInfo(
            on_wait=waits[:_MAX_WAITS],
            on_update=list(si.on_update) if si.on_update else [],
        )
        rest = waits[_MAX_WAITS:]
        while rest:
            extra = nc.sync.nop(nofuse=True)
            extra.ins.sync_info = mybir.SyncInfo(
                on_wait=rest[:_MAX_WAITS], on_update=[])
            rest = rest[_MAX_WAITS:]

    nc.sync.drain()
    nc.all_engine_barrier()
    assert self.sems is not None
    popped = nc._tile_sem_poison_stack.pop()
    assert popped is self._sem_poison
    nc.clear_and_free_semaphores(list(self.sems.allocated().values()))
    nc.all_engine_barrier()


tile_mod.TileContext._drain_and_barrier = _patched_drain_and_barrier

# ----------------------------------------------------------------------------

F32 = mybir.dt.float32
BF16 = mybir.dt.bfloat16
ALU = mybir.AluOpType
ACT_EXP = mybir.ActivationFunctionType.Exp
ACT_SQRT = mybir.ActivationFunctionType.Sqrt

B = 4
TOK = 2048
DIM = 1024
HEADS = 16
DH = 64
HEADS_PC = 8          # heads per core
INNER_PC = HEADS_PC * DH  # 512
NPAIRS = HEADS_PC // 2
QT = 512              # query tile
KC = 128              # key-token chunk
NQT = TOK // QT       # 4
NTT = TOK // KC       # 16
NKD = DIM // 128      # 8
EPS = 1e-5
SCALE = DH ** -0.5


def _ap0(ap, parts):
    """Partition-broadcast AP (stride-0 leading dim) for DMA reads."""
    return bass.AP(tensor=ap.tensor, offset=ap.offset, ap=[[0, parts]] + list(ap.ap))


def _pstride(t, step, n):
    """View tile t's partitions with a stride (partition p -> p*step)."""
    return bass.AP(tensor=t.tensor, offset=t.offset,
                   ap=[[t.ap[0][0] * step, n]] + list(t.ap[1:]))


def build_program(tok=TOK):
    nc = bass.Bass()
    xT = nc.declare_dram_parameter("xT", [DIM, tok], BF16, isOutput=False)
    w = nc.declare_dram_parameter("w", [DIM, 3 * INNER_PC], BF16, isOutput=False)
    seed = nc.declare_dram_parameter("seed", [2, 3 * INNER_PC], BF16, isOutput=False)
    wo = nc.declare_dram_parameter("wo", [INNER_PC, DIM], BF16, isOutput=False)
    mask_d = nc.declare_dram_parameter("mask128", [KC, KC], BF16, isOutput=False)
    eye_d = nc.declare_dram_parameter("eye4", [4, 4], F32, isOutput=False)
    out = nc.declare_dram_parameter("out", [tok, DIM], F32, isOutput=True)
    # DRAM bounce buffers for partition-broadcasts (stride-0 reads are only
    # legal on the DRAM side of a DMA)
    rstd_d = nc.dram_tensor("rstd_row", [1, tok], F32)
    rec_d = nc.dram_tensor("rec_row", [NPAIRS, 8 * QT], BF16)

    with tile_mod.TileContext(nc) as tc, nc.allow_low_precision(
            "bf16 operand tiles; all matmul accumulation stays fp32 PSUM"):
        with (
            tc.tile_pool(name="const", bufs=1) as const,
            tc.tile_pool(name="xt", bufs=NKD) as xtp,
            tc.tile_pool(name="wsb", bufs=NKD) as wp,
            tc.tile_pool(name="wosb", bufs=NPAIRS) as wop,
            tc.tile_pool(name="vpool", bufs=NTT) as vpool,
            tc.tile_pool(name="qkT", bufs=2) as qkp,
            tc.tile_pool(name="osb", bufs=3) as osbp,
            tc.tile_pool(name="oT", bufs=NPAIRS) as oTp,
            tc.tile_pool(name="psb", bufs=3) as ppool,
            tc.tile_pool(name="den", bufs=2) as denp,
            tc.tile_pool(name="sq", bufs=2) as sqp,
            tc.tile_pool(name="out_sb", bufs=4) as outp,
        ):
            # ---------------- constants / big loads ----------------
            ones_col = const.tile([128, 1], BF16, tag="ones_col")
            nc.vector.memset(ones_col, 1.0)
            # sel4: 4 stationary variants [128, 4]; variant v is all-ones in
            # column v, zero elsewhere.  Routes LN stat rows to consecutive
            # PSUM partitions (matmul/DVE PSUM bases must be 32-aligned).
            sel4 = const.tile([128, 16], BF16, tag="sel4")
            nc.vector.memset(sel4, 0.0)
            for v in range(4):
                nc.vector.memset(sel4[:, 5 * v:5 * v + 1], 1.0)
            eps4 = const.tile([4, 1], F32, tag="eps")
            nc.vector.memset(eps4, EPS)
            mask_sb = const.tile([KC, KC], BF16, tag="mask")
            nc.sync.dma_start(out=mask_sb, in_=mask_d[:, :])
            eye4 = const.tile([4, 4], F32, tag="eye4")
            nc.sync.dma_start(out=eye4, in_=eye_d[:, :])
            seed_sb = const.tile([2, 3 * INNER_PC], BF16, tag="seed")
            nc.sync.dma_start(out=seed_sb, in_=seed[:, :])

            # x chunks arrive split in token-halves across four engine
            # queues so the transfers run in parallel and the LN stats start
            # consuming as early as possible.
            xt = []
            qeng = [nc.sync, nc.scalar, nc.gpsimd]
            for kc in range(NKD):
                t = xtp.tile([128, tok], BF16, tag="xt")
                for hf in range(2):
                    hsl = slice(hf * (tok // 2), (hf + 1) * (tok // 2))
                    qeng[(2 * kc + hf) % 3].dma_start(
                        out=t[:, hsl],
                        in_=xT[kc * 128:(kc + 1) * 128, hsl])
                xt.append(t)
            wsb = []
            for kc in range(NKD):
                t = wp.tile([128, 3 * INNER_PC], BF16, tag="wsb")
                nc.sync.dma_start(out=t, in_=w[kc * 128:(kc + 1) * 128, :])
                wsb.append(t)
            wos = []
            for p in range(NPAIRS):
                t = wop.tile([128, DIM], BF16, tag="wosb")
                nc.sync.dma_start(out=t, in_=wo[p * 128:(p + 1) * 128, :])
                wos.append(t)

            # LN-derived rows (device computed, long-lived)
            onm = const.tile([2, tok], BF16, tag="onm")        # r0=std r1=-mu
            bc_sb = const.tile([128, tok], F32, tag="bc")      # rstd bcast
            # NOTE: rstd_col columns are block-permuted: chunk tt lives at
            # column 4*(tt%4) + tt//4 (transpose blocks land contiguously).
            rstd_col = const.tile([128, NTT], F32, tag="rstdc")
            rstd4 = const.tile([4, QT], F32, tag="rstd4")
            std4 = const.tile([4, QT], F32, tag="std4")
            std4b = const.tile([4, QT], BF16, tag="std4b")
            nmu4b = const.tile([4, QT], BF16, tag="nmu4b")
            mu4 = const.tile([4, QT], F32, tag="mu4")
            var4 = const.tile([4, QT], F32, tag="var4")
            musq4 = const.tile([4, QT], F32, tag="musq4")

            # ---------------- phase A: LN stats ----------------
            with (
                tc.tile_pool(name="ps_stats", bufs=1, space="PSUM") as pstat,
            ):
                # Warm-up: dummy matmuls on memset constants (no DMA
                # dependency, so they start at ~0.5us) spanning the input
                # DMA wait, so the HAM clock gate is open (2.4 GHz) when the
                # stats matmuls start.
                warm_ps = pstat.tile([4, 16], F32, tag="warm")
                for _ in range(110):
                    nc.tensor.matmul(out=warm_ps, lhsT=sel4[:, 0:4],
                                     rhs=sel4, start=True, stop=True)
                # Two [4, 512] PSUM tiles: token-slice nt's sum / sum-of-sq
                # rows land on partition nt via the sel4 stationary (other
                # rows accumulate zero), one accumulation group per tile.
                sum_ps = pstat.tile([4, QT], F32, tag="sum")
                sq_ps = pstat.tile([4, QT], F32, tag="sq")
                for kc in range(NKD):
                    for nt in range(NQT):
                        sl = slice(nt * QT, (nt + 1) * QT)
                        first = kc == 0 and nt == 0
                        last = kc == NKD - 1 and nt == NQT - 1
                        sq_t = sqp.tile([128, QT], BF16, tag="sq_t")
                        nc.vector.tensor_mul(sq_t, xt[kc][:, sl], xt[kc][:, sl])
                        nc.tensor.matmul(
                            out=sum_ps, lhsT=sel4[:, 4 * nt:4 * nt + 4],
                            rhs=xt[kc][:, sl],
                            start=first, stop=last)
                        nc.tensor.matmul(
                            out=sq_ps, lhsT=sel4[:, 4 * nt:4 * nt + 4],
                            rhs=sq_t,
                            start=first, stop=last)
                # postproc on [4, 512] (4 lanes)
                nc.vector.tensor_scalar_mul(mu4, sum_ps, 1.0 / DIM)
                nc.vector.tensor_scalar_mul(var4, sq_ps, 1.0 / DIM)
                nc.vector.tensor_mul(musq4, mu4, mu4)
                nc.vector.tensor_sub(var4, var4, musq4)
                nc.scalar.activation(out=std4, in_=var4, func=ACT_SQRT,
                                     bias=eps4, scale=1.0)
                nc.vector.reciprocal(rstd4, std4)
                nc.vector.tensor_copy(std4b, std4)
                nc.vector.tensor_scalar_mul(nmu4b, mu4, -1.0)
                # gather LN rows into operand layouts (cross-shape DMAs:
                # only total element count must match)
                # rstd broadcast [128, tok]: bounce through DRAM, then
                # stride-0 partition-broadcast reads (split per 512-slice so
                # the first Q/K copies unblock as early as possible)
                nc.gpsimd.dma_start(out=rstd_d[0:1, :], in_=rstd4[:, :])
                nc.gpsimd.dma_start(out=onm[0:1, :], in_=std4b[:, :])
                nc.gpsimd.dma_start(out=onm[1:2, :], in_=nmu4b[:, :])
                for j in range(NQT):
                    nc.gpsimd.dma_start(
                        out=bc_sb[:, j * QT:(j + 1) * QT],
                        in_=bass.AP(tensor=rstd_d, offset=j * QT,
                                    ap=[[0, 128], [1, QT]]))
            # ------- phases B-D: projections + attention + out-proj -------
            # One shared [128, 512]-f32 PSUM pool ("proj") serves the QK
            # slices, the V groups, the rstd transposes and the out-proj
            # groups; they never overlap in time.  8 banks total:
            # proj 2 + scores 4 + o_ps 2.
            v_sb = [None] * NTT
            oTs = []
            with (
                tc.tile_pool(name="ps_proj", bufs=2, space="PSUM") as pproj,
                tc.tile_pool(name="ps_s", bufs=2, space="PSUM") as pss,
                tc.tile_pool(name="ps_o", bufs=2, space="PSUM") as pso,
            ):
                def emit_qk_slice(p, dst, d, nt):
                    """One [128, 512] token-slice of the Q or K projection of
                    pair p (d=0 -> Q, d=1 -> K).  Seed matmul last so the
                    group never waits on the LN postprocessing."""
                    sl = slice(nt * QT, (nt + 1) * QT)
                    cofs = d * INNER_PC + p * 128
                    ps = pproj.tile([128, QT], F32, tag="proj", name="qk_ps")
                    for kc in range(NKD):
                        nc.tensor.matmul(
                            out=ps, lhsT=wsb[kc][:, cofs:cofs + 128],
                            rhs=xt[kc][:, sl],
                            start=(kc == 0), stop=False)
                    nc.tensor.matmul(
                        out=ps, lhsT=seed_sb[:, cofs:cofs + 128],
                        rhs=onm[:, sl], start=False, stop=True)
                    nc.vector.tensor_mul(dst[:, sl], ps, bc_sb[:, sl])

                def emit_rstd_col():
                    # 4 PE transposes of [4, 128] blocks; block g lands at
                    # contiguous cols [4g, 4g+4) (permuted order, see above).
                    rc_ps = pproj.tile([128, QT], F32, tag="proj", name="rc")
                    for g in range(4):
                        nc.tensor.transpose(
                            out=rc_ps[:, 4 * g:4 * (g + 1)],
                            in_=rstd4[:, g * 128:(g + 1) * 128],
                            identity=eye4)
                    nc.vector.tensor_copy(rstd_col, rc_ps[:, 0:NTT])

                def emit_v_group(tt):
                    tsl = slice(tt * KC, (tt + 1) * KC)
                    v_ps = pproj.tile([128, INNER_PC], F32, tag="proj",
                                      name="v_ps")
                    for kc in range(NKD):
                        nc.tensor.matmul(
                            out=v_ps, lhsT=xt[kc][:, tsl],
                            rhs=wsb[kc][:, 2 * INNER_PC:3 * INNER_PC],
                            start=(kc == 0), stop=False)
                    nc.tensor.matmul(
                        out=v_ps, lhsT=onm[:, tsl],
                        rhs=seed_sb[:, 2 * INNER_PC:3 * INNER_PC],
                        start=False, stop=True)
                    vt = vpool.tile([128, HEADS_PC * (DH + 1)], BF16,
                                    tag="v_sb", name=f"v_sb{tt}")
                    v3 = vt.rearrange("p (h w) -> p h w", w=DH + 1)
                    pc = 4 * (tt % 4) + tt // 4  # permuted rstd_col index
                    nc.vector.tensor_scalar(
                        out=v3[:, :, 0:DH],
                        in0=v_ps.rearrange("p (h w) -> p h w", w=DH),
                        scalar1=rstd_col[:, pc:pc + 1], scalar2=None,
                        op0=ALU.mult)
                    nc.vector.memset(v3[:, :, DH:DH + 1], 1.0)
                    v_sb[tt] = vt

                def emit_outproj_tt(tt):
                    tsl = slice(tt * KC, (tt + 1) * KC)
                    for nb in range(DIM // QT):
                        nsl = slice(nb * QT, (nb + 1) * QT)
                        ps = pproj.tile([128, QT], F32, tag="proj",
                                        name="out_ps")
                        for p in range(NPAIRS):
                            nc.tensor.matmul(
                                out=ps, lhsT=oTs[p][:, tsl],
                                rhs=wos[p][:, nsl],
                                start=(p == 0), stop=(p == NPAIRS - 1))
                        ob = outp.tile([128, QT], F32, tag="out_sb")
                        nc.vector.tensor_copy(ob, ps)
                        nc.sync.dma_start(out=out[tsl, nsl], in_=ob)

                def emit_attn_qtile(p, t_i, qT, kT, oT):
                    """Scores/exp/mask/PV for one query tile, then the
                    per-qtile denominator chain and normalize into oT."""
                    qsl0 = t_i * QT
                    nch = (t_i + 1) * QT // KC
                    o_ps = [pso.tile([DH + 1, QT], F32, tag="o_ps",
                                     name=f"o_ps{p}_{t_i}_{h}")
                            for h in range(2)]
                    p_tiles = {}

                    def emit_scores(c):
                        m = c - (nch - 4)
                        lo = 128 * m if m > 0 else 0
                        csl = slice(c * KC, (c + 1) * KC)
                        s_ps = pss.tile([128, 2 * QT], F32, tag="s_ps")
                        p_sb = ppool.tile([128, 2 * QT], BF16, tag="p_sb")
                        for h in range(2):
                            nc.tensor.matmul(
                                out=s_ps[:, h * QT + lo:(h + 1) * QT],
                                lhsT=kT[h * DH:(h + 1) * DH, csl],
                                rhs=qT[h * DH:(h + 1) * DH,
                                       qsl0 + lo:qsl0 + QT],
                                start=True, stop=True)
                        s3 = s_ps.rearrange("p (h q) -> p h q", q=QT)
                        p3 = p_sb.rearrange("p (h q) -> p h q", q=QT)
                        nc.scalar.activation(
                            out=p3[:, :, lo:QT], in_=s3[:, :, lo:QT],
                            func=ACT_EXP, scale=SCALE)
                        if m >= 0:
                            for h in range(2):
                                nc.vector.tensor_mul(
                                    p_sb[:, h * QT + lo:h * QT + lo + KC],
                                    p_sb[:, h * QT + lo:h * QT + lo + KC],
                                    mask_sb)
                        p_tiles[c] = p_sb

                    def emit_pv(c):
                        m = c - (nch - 4)
                        lo = 128 * m if m > 0 else 0
                        p_sb = p_tiles.pop(c)
                        for h in range(2):
                            hc = (2 * p + h) * (DH + 1)
                            nc.tensor.matmul(
                                out=o_ps[h][:, lo:QT],
                                lhsT=v_sb[c][:, hc:hc + DH + 1],
                                rhs=p_sb[:, h * QT + lo:(h + 1) * QT],
                                start=(c == 0), stop=(c == nch - 1),
                                skip_group_check=True)

                    emit_scores(0)
                    for c in range(1, nch):
                        emit_scores(c)
                        emit_pv(c - 1)
                    emit_pv(nch - 1)
                    # free PSUM fast (O^T rows + denominator row 64)
                    o_sb = [osbp.tile([DH + 1, QT], BF16, tag=f"o_sb{h}",
                                      name=f"o_sb{p}_{t_i}_{h}")
                            for h in range(2)]
                    for h in range(2):
                        nc.vector.tensor_copy(o_sb[h], o_ps[h])
                    # per-qtile denominator chain: den16 partition 8h+j
                    # holds tokens [64j, 64j+64) of head h; flattened
                    # partition-major this gives rec_d offsets 512h+64j+e.
                    den16 = denp.tile([16, 64], BF16, tag="den16")
                    rec16 = denp.tile([16, 64], BF16, tag="rec16")
                    rb_q = denp.tile([64, 2 * QT], BF16, tag="rb_q")
                    for h in range(2):
                        nc.gpsimd.dma_start(
                            out=den16[8 * h:8 * h + 8, :],
                            in_=o_sb[h][DH:DH + 1, :])
                    nc.vector.reciprocal(rec16, den16)
                    dofs = p * 8 * QT + t_i * 2 * QT
                    nc.gpsimd.dma_start(
                        out=bass.AP(tensor=rec_d, offset=dofs,
                                    ap=[[2 * QT, 1], [1, 2 * QT]]),
                        in_=rec16[:, :])
                    nc.gpsimd.dma_start(
                        out=rb_q,
                        in_=bass.AP(tensor=rec_d, offset=dofs,
                                    ap=[[0, 64], [1, 2 * QT]]))
                    qsl = slice(qsl0, qsl0 + QT)
                    for h in range(2):
                        nc.vector.tensor_mul(
                            oT[h * DH:(h + 1) * DH, qsl],
                            o_sb[h][0:DH, :],
                            rb_q[:, h * QT:(h + 1) * QT])

                # ---- schedule ----
                # V groups lead: their PSUM-freeing copies gate only on
                # rstd_col (ready right after the postproc), while the q/k
                # copies wait for the slower bc broadcast chain — so V in
                # front keeps the proj pool cycling during the LN latency.
                qT = qkp.tile([128, tok], BF16, tag="qT", name="qT0")
                kT = qkp.tile([128, tok], BF16, tag="kT", name="kT0")
                emit_rstd_col()
                for tt in range(2):
                    emit_v_group(tt)
                emit_qk_slice(0, qT, 0, 0)
                emit_qk_slice(0, kT, 1, 0)
                for tt in range(2, 6):
                    emit_v_group(tt)

                for p in range(NPAIRS):
                    oT = oTp.tile([128, tok], BF16, tag="oT", name=f"oT{p}")
                    oTs.append(oT)
                    nxt = []
                    if p + 1 < NPAIRS:
                        qT2 = qkp.tile([128, tok], BF16, tag="qT",
                                       name=f"qT{p + 1}")
                        kT2 = qkp.tile([128, tok], BF16, tag="kT",
                                       name=f"kT{p + 1}")
                        nxt = [(p + 1, dst, d, nt)
                               for d, dst in ((0, qT2), (1, kT2))
                               for nt in range(NQT)]
                    for t_i in range(NQT):
                        emit_attn_qtile(p, t_i, qT, kT, oT)
                        if p == 0 and t_i < 3:
                            # just-in-time rest of pair 0's QK and V
                            # (V groups 0-5 were emitted up front)
                            emit_qk_slice(0, qT, 0, t_i + 1)
                            emit_qk_slice(0, kT, 1, t_i + 1)
                            for tt in range(4 * t_i + 6, 4 * t_i + 10):
                                if tt < NTT:
                                    emit_v_group(tt)
                        if p == NPAIRS - 1 and t_i < NQT - 1:
                            # out-proj for this qtile's tokens rides along
                            # (the last qtile's blocks run after the pools
                            # close, pairs 0-2 first, so they overlap the
                            # final denominator chain)
                            for tt in range(4 * t_i, 4 * (t_i + 1)):
                                emit_outproj_tt(tt)
                        for _ in range(2):
                            if nxt:
                                emit_qk_slice(*nxt.pop(0))
                    while nxt:
                        emit_qk_slice(*nxt.pop(0))
                    if p + 1 < NPAIRS:
                        qT, kT = qT2, kT2

            # Final out-proj blocks (tokens of the last qtile): pairs 0-2
            # accumulate while pair 3's last denominator chain completes,
            # the pair-3 matmul joins last.
            with tc.tile_pool(name="ps_fin", bufs=6, space="PSUM") as pfin:
                fin = [(tt, nb) for tt in range(4 * (NQT - 1), NTT)
                       for nb in range(DIM // QT)]
                tiles = {}

                def fin_p012(i):
                    tt, nb = fin[i]
                    ps = pfin.tile([128, QT], F32, tag="fin",
                                   name=f"fin{tt}_{nb}")
                    for p in range(NPAIRS - 1):
                        nc.tensor.matmul(
                            out=ps, lhsT=oTs[p][:, tt * KC:(tt + 1) * KC],
                            rhs=wos[p][:, nb * QT:(nb + 1) * QT],
                            start=(p == 0), stop=False)
                    tiles[i] = ps

                for i in range(6):
                    fin_p012(i)
                for i in range(len(fin)):
                    if i >= 6:
                        fin_p012(i)
                    tt, nb = fin[i]
                    nc.tensor.matmul(
                        out=tiles[i],
                        lhsT=oTs[NPAIRS - 1][:, tt * KC:(tt + 1) * KC],
                        rhs=wos[NPAIRS - 1][:, nb * QT:(nb + 1) * QT],
                        start=False, stop=True)
                    ob = outp.tile([128, QT], F32, tag="out_sb")
                    # the exp stream is over: split the copies over DVE and
                    # ACT, and the final 512KB writes over all three DMA
                    # queues so the drain doesn't serialize on one queue
                    if i % 2 == 0:
                        nc.vector.tensor_copy(ob, tiles.pop(i))
                    else:
                        nc.scalar.copy(ob, tiles.pop(i))
                    [nc.sync, nc.scalar, nc.gpsimd][i % 3].dma_start(
                        out=out[tt * KC:(tt + 1) * KC,
                                nb * QT:(nb + 1) * QT], in_=ob)

    return nc


def make_masks():
    import ml_dtypes

    k = np.arange(KC)[:, None]
    q = np.arange(KC)[None, :]
    return (q >= k).astype(ml_dtypes.bfloat16)


def make_in_maps(x, ln_gamma, ln_beta, w_qkv, w_out):
    import ml_dtypes

    bf16 = ml_dtypes.bfloat16
    x = np.asarray(x, np.float32)
    g_ = np.asarray(ln_gamma, np.float32)
    b_ = np.asarray(ln_beta, np.float32)
    w_qkv = np.asarray(w_qkv, np.float32)
    w_out = np.asarray(w_out, np.float32)
    mask128 = make_masks()
    eye4 = np.eye(4, dtype=np.float32)
    in_maps = []
    for c in range(8):
        b = c // 2
        g = c % 2
        cs = slice(g * INNER_PC, (g + 1) * INNER_PC)
        Wraw = np.concatenate(
            [w_qkv[:, 0 * DIM:1 * DIM][:, cs],
             w_qkv[:, 1 * DIM:2 * DIM][:, cs],
             w_qkv[:, 2 * DIM:3 * DIM][:, cs]], axis=1)
        Wp = (Wraw * g_[:, None]).astype(bf16)
        seed = np.stack([b_ @ Wraw,
                         Wp.astype(np.float32).sum(axis=0)]).astype(bf16)
        in_maps.append({
            "xT": np.ascontiguousarray(x[b].T).astype(bf16),
            "w": np.ascontiguousarray(Wp),
            "seed": seed,
            "wo": np.ascontiguousarray(w_out[cs, :]).astype(bf16),
            "mask128": mask128,
            "eye4": eye4,
        })
    return in_maps


_PROG = None


def kernel(x, ln_gamma, ln_beta, w_qkv, w_out):
    global _PROG
    from concourse.bass_utils import run_bass_kernel_spmd

    if _PROG is None:
        _PROG = build_program(TOK)
    in_maps = make_in_maps(x, ln_gamma, ln_beta, w_qkv, w_out)
    res = run_bass_kernel_spmd(_PROG, in_maps, list(range(8)))
    parts = [res.results[c]["out"] for c in range(8)]
    out = np.empty((B, TOK, DIM), np.float32)
    for b in range(B):
        out[b] = parts[2 * b] + parts[2 * b + 1]
    return out


# revision 50
# speedup vs baseline: 1.1615x; 1.0151x over previous
"""Causal attention block (LN -> QKV -> causal MHA -> out-proj) on 8 trn2
NeuronCores via Bass/Tile.

Sharding: core c handles batch b=c//2 and head-group g=c%2 (8 of 16 heads).
Data parallel over batch, tensor parallel over heads; the out-proj partial
sums (2 per batch) are reduced on the host during the gather, so the device
program needs no collectives and is pure SPMD.

v2 design (bf16 everywhere, PE kept warm, no DRAM roundtrips):
  - all matmul operands bf16 (full-rate 1 cyc/row, FWL weight loads, half
    the DMA); PSUM accumulation stays fp32.
  - LN stats land in a [4, 512] PSUM layout so the postprocessing runs on
    4 partitions instead of 1; rstd is folded into the PSUM->SBUF copies
    (per-partition tensor_scalar for V, a broadcast-row multiply for Q/K),
    so x itself is never rewritten.  The mean/beta correction enters each
    projection as a K=2 seed matmul with rows [std, -mu].
  - scores are computed transposed per 128-key chunk with 2 heads packed;
    exp runs on ACT straight out of PSUM with the 1/sqrt(dh) scale folded
    in; diagonal chunks are narrowed to the valid q-range (saves PE + ACT)
    and only the 128x128 boundary block gets a 0/1 mask multiply.
  - softmax denominators ride as a 65th ones-column of V through the P@V
    matmul; per pair they are DMA-gathered into a [64, 64] tile for one
    batched reciprocal, spread back with two small DMAs, and applied in the
    normalize copy into the SBUF-resident O^T (no DRAM scratch).
  - the QK projection of pair p+1 is interleaved into the (ACT-bound)
    attention stream of pair p so the PE never idles long enough to be
    clock-throttled.
"""

import numpy as np

import concourse.bass as bass
import concourse.mybir as mybir
import concourse.tile as tile_mod

# ----------------------------------------------------------------------------
# Workaround for this walrus build rejecting instructions that carry more than
# MAX_WAITS semaphore waits ("Too many sync wait commands" in CoreV3GenImpl
# setupSyncWait — hit on Drain and Matmult/S3_LW encodings). Split excess
# waits onto single-wait NOP carrier instructions emitted just before the
# original instruction on the same engine: program order on the sequencer
# makes this semantically identical (waits are AND conditions).
# ----------------------------------------------------------------------------
_MAX_WAITS = 1
_orig_add_instruction = tile_mod.TileContext._add_instruction
_carrier_id = [0]


def _split_waits_add_instruction(self, inst):
    si = inst.sync_info
    if (
        si is not None
        and si.on_wait
        and len(si.on_wait) > _MAX_WAITS
        and inst.engine != mybir.EngineType.Unassigned
    ):
        waits = list(si.on_wait)
        keep = waits[-_MAX_WAITS:]
        for w in waits[:-_MAX_WAITS]:
            _carrier_id[0] += 1
            nop = mybir.InstNoOp(name=f"I-waitc-{_carrier_id[0]}")
            nop.engine = inst.engine
            nop.sync_info = mybir.SyncInfo(on_wait=[w], on_update=[])
            _orig_add_instruction(self, nop)
        inst.sync_info = mybir.SyncInfo(
            on_wait=keep,
            on_update=list(si.on_update) if si.on_update else [],
        )
    _orig_add_instruction(self, inst)


tile_mod.TileContext._add_instruction = _split_waits_add_instruction

from concourse.vector_clock import ScopedClock


def _patched_drain_and_barrier(self, tick_clock, wait_clock):
    # Same wait-splitting for the TileContext exit drain, which is emitted
    # after lowering (outside _add_instruction).
    nc = self.nc
    carrier = nc.sync.nop(nofuse=True)
    wait_clock.add_sem_waits(carrier.ins, ScopedClock({None: tick_clock.global_clock}))
    si = carrier.ins.sync_info
    waits = list(si.on_wait) if si is not None and si.on_wait else []
    if len(waits) > _MAX_WAITS:
        carrier.ins.sync_info = mybir.SyncInfo(
            on_wait=waits[:_MAX_WAITS],
            on_update=list(si.on_update) if si.on_update else [],
        )
        rest = waits[_MAX_WAITS:]
        while rest:
            extra = nc.sync.nop(nofuse=True)
            extra.ins.sync_info = mybir.SyncInfo(
                on_wait=rest[:_MAX_WAITS], on_update=[])
            rest = rest[_MAX_WAITS:]

    nc.sync.drain()
    nc.all_engine_barrier()
    assert self.sems is not None
    popped = nc._tile_sem_poison_stack.pop()
    assert popped is self._sem_poison
    nc.clear_and_free_semaphores(list(self.sems.allocated().values()))
    nc.all_engine_barrier()


tile_mod.TileContext._drain_and_barrier = _patched_drain_and_barrier

# ----------------------------------------------------------------------------

F32 = mybir.dt.float32
BF16 = mybir.dt.bfloat16
ALU = mybir.AluOpType
ACT_EXP = mybir.ActivationFunctionType.Exp
ACT_SQRT = mybir.ActivationFunctionType.Sqrt

B = 4
TOK = 2048
DIM = 1024
HEADS = 16
DH = 64
HEADS_PC = 8          # heads per core
INNER_PC = HEADS_PC * DH  # 512
NPAIRS = HEADS_PC // 2
QT = 512              # query tile
KC = 128              # key-token chunk
NQT = TOK // QT       # 4
NTT = TOK // KC       # 16
NKD = DIM // 128      # 8
EPS = 1e-5
SCALE = DH ** -0.5


def _ap0(ap, parts):
    """Partition-broadcast AP (stride-0 leading dim) for DMA reads."""
    return bass.AP(tensor=ap.tensor, offset=ap.offset, ap=[[0, parts]] + list(ap.ap))


def _pstride(t, step, n):
    """View tile t's partitions with a stride (partition p -> p*step)."""
    return bass.AP(tensor=t.tensor, offset=t.offset,
                   ap=[[t.ap[0][0] * step, n]] + list(t.ap[1:]))


def build_program(tok=TOK):
    nc = bass.Bass()
    xT = nc.declare_dram_parameter("xT", [DIM, tok], BF16, isOutput=False)
    w = nc.declare_dram_parameter("w", [DIM, 3 * INNER_PC], BF16, isOutput=False)
    seed = nc.declare_dram_parameter("seed", [2, 3 * INNER_PC], BF16, isOutput=False)
    wo = nc.declare_dram_parameter("wo", [INNER_PC, DIM], BF16, isOutput=False)
    mask_d = nc.declare_dram_parameter("mask128", [KC, KC], BF16, isOutput=False)
    eye_d = nc.declare_dram_parameter("eye4", [4, 4], F32, isOutput=False)
    out = nc.declare_dram_parameter("out", [tok, DIM], F32, isOutput=True)
    # DRAM bounce buffers for partition-broadcasts (stride-0 reads are only
    # legal on the DRAM side of a DMA)
    rstd_d = nc.dram_tensor("rstd_row", [1, tok], F32)
    rec_d = nc.dram_tensor("rec_row", [NPAIRS, 8 * QT], BF16)

    with tile_mod.TileContext(nc) as tc, nc.allow_low_precision(
            "bf16 operand tiles; all matmul accumulation stays fp32 PSUM"):
        with (
            tc.tile_pool(name="const", bufs=1) as const,
            tc.tile_pool(name="xt", bufs=NKD) as xtp,
            tc.tile_pool(name="wsb", bufs=NKD) as wp,
            tc.tile_pool(name="wosb", bufs=NPAIRS) as wop,
            tc.tile_pool(name="vpool", bufs=NTT) as vpool,
            tc.tile_pool(name="qkT", bufs=2) as qkp,
            tc.tile_pool(name="osb", bufs=3) as osbp,
            tc.tile_pool(name="oT", bufs=NPAIRS) as oTp,
            tc.tile_pool(name="psb", bufs=3) as ppool,
            tc.tile_pool(name="den", bufs=2) as denp,
            tc.tile_pool(name="sq", bufs=2) as sqp,
            tc.tile_pool(name="out_sb", bufs=6) as outp,
        ):
            # ---------------- constants / big loads ----------------
            ones_col = const.tile([128, 1], BF16, tag="ones_col")
            nc.vector.memset(ones_col, 1.0)
            # sel4: 4 stationary variants [128, 4]; variant v is all-ones in
            # column v, zero elsewhere.  Routes LN stat rows to consecutive
            # PSUM partitions (matmul/DVE PSUM bases must be 32-aligned).
            sel4 = const.tile([128, 16], BF16, tag="sel4")
            nc.vector.memset(sel4, 0.0)
            for v in range(4):
                nc.vector.memset(sel4[:, 5 * v:5 * v + 1], 1.0)
            eps4 = const.tile([4, 1], F32, tag="eps")
            nc.vector.memset(eps4, EPS)
            mask_sb = const.tile([KC, KC], BF16, tag="mask")
            nc.sync.dma_start(out=mask_sb, in_=mask_d[:, :])
            eye4 = const.tile([4, 4], F32, tag="eye4")
            nc.sync.dma_start(out=eye4, in_=eye_d[:, :])
            seed_sb = const.tile([2, 3 * INNER_PC], BF16, tag="seed")
            nc.sync.dma_start(out=seed_sb, in_=seed[:, :])

            # x chunks arrive split in token-halves across four engine
            # queues so the transfers run in parallel and the LN stats start
            # consuming as early as possible.
            xt = []
            qeng = [nc.sync, nc.scalar, nc.gpsimd]
            for kc in range(NKD):
                t = xtp.tile([128, tok], BF16, tag="xt")
                for hf in range(2):
                    hsl = slice(hf * (tok // 2), (hf + 1) * (tok // 2))
                    qeng[(2 * kc + hf) % 3].dma_start(
                        out=t[:, hsl],
                        in_=xT[kc * 128:(kc + 1) * 128, hsl])
                xt.append(t)
            wsb = []
            for kc in range(NKD):
                t = wp.tile([128, 3 * INNER_PC], BF16, tag="wsb")
                nc.sync.dma_start(out=t, in_=w[kc * 128:(kc + 1) * 128, :])
                wsb.append(t)
            wos = []
            for p in range(NPAIRS):
                t = wop.tile([128, DIM], BF16, tag="wosb")
                nc.sync.dma_start(out=t, in_=wo[p * 128:(p + 1) * 128, :])
                wos.append(t)

            # LN-derived rows (device computed, long-lived)
            onm = const.tile([2, tok], BF16, tag="onm")        # r0=std r1=-mu
            bc_sb = const.tile([128, tok], F32, tag="bc")      # rstd bcast
            # NOTE: rstd_col columns are block-permuted: chunk tt lives at
            # column 4*(tt%4) + tt//4 (transpose blocks land contiguously).
            rstd_col = const.tile([128, NTT], F32, tag="rstdc")
            rstd4 = const.tile([4, QT], F32, tag="rstd4")
            std4 = const.tile([4, QT], F32, tag="std4")
            std4b = const.tile([4, QT], BF16, tag="std4b")
            nmu4b = const.tile([4, QT], BF16, tag="nmu4b")
            mu4 = const.tile([4, QT], F32, tag="mu4")
            var4 = const.tile([4, QT], F32, tag="var4")
            musq4 = const.tile([4, QT], F32, tag="musq4")

            # ---------------- phase A: LN stats ----------------
            with (
                tc.tile_pool(name="ps_stats", bufs=1, space="PSUM") as pstat,
            ):
                # Warm-up: dummy matmuls on memset constants (no DMA
                # dependency, so they start at ~0.5us) spanning the input
                # DMA wait, so the HAM clock gate is open (2.4 GHz) when the
                # stats matmuls start.
                warm_ps = pstat.tile([4, 16], F32, tag="warm")
                for _ in range(110):
                    nc.tensor.matmul(out=warm_ps, lhsT=sel4[:, 0:4],
                                     rhs=sel4, start=True, stop=True)
                # Two [4, 512] PSUM tiles: token-slice nt's sum / sum-of-sq
                # rows land on partition nt via the sel4 stationary (other
                # rows accumulate zero), one accumulation group per tile.
                sum_ps = pstat.tile([4, QT], F32, tag="sum")
                sq_ps = pstat.tile([4, QT], F32, tag="sq")
                for kc in range(NKD):
                    for nt in range(NQT):
                        sl = slice(nt * QT, (nt + 1) * QT)
                        first = kc == 0 and nt == 0
                        last = kc == NKD - 1 and nt == NQT - 1
                        sq_t = sqp.tile([128, QT], BF16, tag="sq_t")
                        nc.vector.tensor_mul(sq_t, xt[kc][:, sl], xt[kc][:, sl])
                        nc.tensor.matmul(
                            out=sum_ps, lhsT=sel4[:, 4 * nt:4 * nt + 4],
                            rhs=xt[kc][:, sl],
                            start=first, stop=last)
                        nc.tensor.matmul(
                            out=sq_ps, lhsT=sel4[:, 4 * nt:4 * nt + 4],
                            rhs=sq_t,
                            start=first, stop=last)
                # postproc on [4, 512] (4 lanes)
                nc.vector.tensor_scalar_mul(mu4, sum_ps, 1.0 / DIM)
                nc.vector.tensor_scalar_mul(var4, sq_ps, 1.0 / DIM)
                nc.vector.tensor_mul(musq4, mu4, mu4)
                nc.vector.tensor_sub(var4, var4, musq4)
                nc.scalar.activation(out=std4, in_=var4, func=ACT_SQRT,
                                     bias=eps4, scale=1.0)
                nc.vector.reciprocal(rstd4, std4)
                nc.vector.tensor_copy(std4b, std4)
                nc.vector.tensor_scalar_mul(nmu4b, mu4, -1.0)
                # gather LN rows into operand layouts (cross-shape DMAs:
                # only total element count must match)
                # rstd broadcast [128, tok]: bounce through DRAM, then
                # stride-0 partition-broadcast reads (split per 512-slice so
                # the first Q/K copies unblock as early as possible)
                nc.gpsimd.dma_start(out=rstd_d[0:1, :], in_=rstd4[:, :])
                nc.gpsimd.dma_start(out=onm[0:1, :], in_=std4b[:, :])
                nc.gpsimd.dma_start(out=onm[1:2, :], in_=nmu4b[:, :])
                for j in range(NQT):
                    nc.gpsimd.dma_start(
                        out=bc_sb[:, j * QT:(j + 1) * QT],
                        in_=bass.AP(tensor=rstd_d, offset=j * QT,
                                    ap=[[0, 128], [1, QT]]))
            # ------- phases B-D: projections + attention + out-proj -------
            # One shared [128, 512]-f32 PSUM pool ("proj") serves the QK
            # slices, the V groups, the rstd transposes and the out-proj
            # groups; they never overlap in time.  8 banks total:
            # proj 2 + scores 4 + o_ps 2.
            v_sb = [None] * NTT
            oTs = []
            with (
                tc.tile_pool(name="ps_proj", bufs=2, space="PSUM") as pproj,
                tc.tile_pool(name="ps_s", bufs=2, space="PSUM") as pss,
                tc.tile_pool(name="ps_o", bufs=2, space="PSUM") as pso,
            ):
                def emit_qk_slice(p, dst, d, nt):
                    """One [128, 512] token-slice of the Q or K projection of
                    pair p (d=0 -> Q, d=1 -> K).  Seed matmul last so the
                    group never waits on the LN postprocessing."""
                    sl = slice(nt * QT, (nt + 1) * QT)
                    cofs = d * INNER_PC + p * 128
                    ps = pproj.tile([128, QT], F32, tag="proj", name="qk_ps")
                    for kc in range(NKD):
                        nc.tensor.matmul(
                            out=ps, lhsT=wsb[kc][:, cofs:cofs + 128],
                            rhs=xt[kc][:, sl],
                            start=(kc == 0), stop=False)
                    nc.tensor.matmul(
                        out=ps, lhsT=seed_sb[:, cofs:cofs + 128],
                        rhs=onm[:, sl], start=False, stop=True)
                    nc.vector.tensor_mul(dst[:, sl], ps, bc_sb[:, sl])

                def emit_rstd_col():
                    # 4 PE transposes of [4, 128] blocks; block g lands at
                    # contiguous cols [4g, 4g+4) (permuted order, see above).
                    rc_ps = pproj.tile([128, QT], F32, tag="proj", name="rc")
                    for g in range(4):
                        nc.tensor.transpose(
                            out=rc_ps[:, 4 * g:4 * (g + 1)],
                            in_=rstd4[:, g * 128:(g + 1) * 128],
                            identity=eye4)
                    nc.vector.tensor_copy(rstd_col, rc_ps[:, 0:NTT])

                def emit_v_group(tt):
                    tsl = slice(tt * KC, (tt + 1) * KC)
                    v_ps = pproj.tile([128, INNER_PC], F32, tag="proj",
                                      name="v_ps")
                    for kc in range(NKD):
                        nc.tensor.matmul(
                            out=v_ps, lhsT=xt[kc][:, tsl],
                            rhs=wsb[kc][:, 2 * INNER_PC:3 * INNER_PC],
                            start=(kc == 0), stop=False)
                    nc.tensor.matmul(
                        out=v_ps, lhsT=onm[:, tsl],
                        rhs=seed_sb[:, 2 * INNER_PC:3 * INNER_PC],
                        start=False, stop=True)
                    vt = vpool.tile([128, HEADS_PC * (DH + 1)], BF16,
                                    tag="v_sb", name=f"v_sb{tt}")
                    v3 = vt.rearrange("p (h w) -> p h w", w=DH + 1)
                    pc = 4 * (tt % 4) + tt // 4  # permuted rstd_col index
                    nc.vector.tensor_scalar(
                        out=v3[:, :, 0:DH],
                        in0=v_ps.rearrange("p (h w) -> p h w", w=DH),
                        scalar1=rstd_col[:, pc:pc + 1], scalar2=None,
                        op0=ALU.mult)
                    nc.vector.memset(v3[:, :, DH:DH + 1], 1.0)
                    v_sb[tt] = vt

                def emit_outproj_tt(tt):
                    tsl = slice(tt * KC, (tt + 1) * KC)
                    for nb in range(DIM // QT):
                        nsl = slice(nb * QT, (nb + 1) * QT)
                        ps = pproj.tile([128, QT], F32, tag="proj",
                                        name="out_ps")
                        for p in range(NPAIRS):
                            nc.tensor.matmul(
                                out=ps, lhsT=oTs[p][:, tsl],
                                rhs=wos[p][:, nsl],
                                start=(p == 0), stop=(p == NPAIRS - 1))
                        ob = outp.tile([128, QT], F32, tag="out_sb")
                        nc.vector.tensor_copy(ob, ps)
                        nc.sync.dma_start(out=out[tsl, nsl], in_=ob)

                def emit_attn_qtile(p, t_i, qT, kT, oT):
                    """Scores/exp/mask/PV for one query tile, then the
                    per-qtile denominator chain and normalize into oT."""
                    qsl0 = t_i * QT
                    nch = (t_i + 1) * QT // KC
                    o_ps = [pso.tile([DH + 1, QT], F32, tag="o_ps",
                                     name=f"o_ps{p}_{t_i}_{h}")
                            for h in range(2)]
                    p_tiles = {}

                    def emit_scores(c):
                        m = c - (nch - 4)
                        lo = 128 * m if m > 0 else 0
                        csl = slice(c * KC, (c + 1) * KC)
                        s_ps = pss.tile([128, 2 * QT], F32, tag="s_ps")
                        p_sb = ppool.tile([128, 2 * QT], BF16, tag="p_sb")
                        for h in range(2):
                            nc.tensor.matmul(
                                out=s_ps[:, h * QT + lo:(h + 1) * QT],
                                lhsT=kT[h * DH:(h + 1) * DH, csl],
                                rhs=qT[h * DH:(h + 1) * DH,
                                       qsl0 + lo:qsl0 + QT],
                                start=True, stop=True)
                        s3 = s_ps.rearrange("p (h q) -> p h q", q=QT)
                        p3 = p_sb.rearrange("p (h q) -> p h q", q=QT)
                        nc.scalar.activation(
                            out=p3[:, :, lo:QT], in_=s3[:, :, lo:QT],
                            func=ACT_EXP, scale=SCALE)
                        if m >= 0:
                            for h in range(2):
                                nc.vector.tensor_mul(
                                    p_sb[:, h * QT + lo:h * QT + lo + KC],
                                    p_sb[:, h * QT + lo:h * QT + lo + KC],
                                    mask_sb)
                        p_tiles[c] = p_sb

                    def emit_pv(c):
                        m = c - (nch - 4)
                        lo = 128 * m if m > 0 else 0
                        p_sb = p_tiles.pop(c)
                        for h in range(2):
                            hc = (2 * p + h) * (DH + 1)
                            nc.tensor.matmul(
                                out=o_ps[h][:, lo:QT],
                                lhsT=v_sb[c][:, hc:hc + DH + 1],
                                rhs=p_sb[:, h * QT + lo:(h + 1) * QT],
                                start=(c == 0), stop=(c == nch - 1),
                                skip_group_check=True)

                    emit_scores(0)
                    for c in range(1, nch):
                        emit_scores(c)
                        emit_pv(c - 1)
                    emit_pv(nch - 1)
                    # free PSUM fast (O^T rows + denominator row 64)
                    o_sb = [osbp.tile([DH + 1, QT], BF16, tag=f"o_sb{h}",
                                      name=f"o_sb{p}_{t_i}_{h}")
                            for h in range(2)]
                    for h in range(2):
                        nc.vector.tensor_copy(o_sb[h], o_ps[h])
                    # per-qtile denominator chain: den16 partition 8h+j
                    # holds tokens [64j, 64j+64) of head h; flattened
                    # partition-major this gives rec_d offsets 512h+64j+e.
                    den16 = denp.tile([16, 64], BF16, tag="den16")
                    rec16 = denp.tile([16, 64], BF16, tag="rec16")
                    rb_q = denp.tile([64, 2 * QT], BF16, tag="rb_q")
                    for h in range(2):
                        nc.gpsimd.dma_start(
                            out=den16[8 * h:8 * h + 8, :],
                            in_=o_sb[h][DH:DH + 1, :])
                    nc.vector.reciprocal(rec16, den16)
                    dofs = p * 8 * QT + t_i * 2 * QT
                    nc.gpsimd.dma_start(
                        out=bass.AP(tensor=rec_d, offset=dofs,
                                    ap=[[2 * QT, 1], [1, 2 * QT]]),
                        in_=rec16[:, :])
                    nc.gpsimd.dma_start(
                        out=rb_q,
                        in_=bass.AP(tensor=rec_d, offset=dofs,
                                    ap=[[0, 64], [1, 2 * QT]]))
                    qsl = slice(qsl0, qsl0 + QT)
                    for h in range(2):
                        nc.vector.tensor_mul(
                            oT[h * DH:(h + 1) * DH, qsl],
                            o_sb[h][0:DH, :],
                            rb_q[:, h * QT:(h + 1) * QT])

                # ---- schedule ----
                # V groups lead: their PSUM-freeing copies gate only on
                # rstd_col (ready right after the postproc), while the q/k
                # copies wait for the slower bc broadcast chain — so V in
                # front keeps the proj pool cycling during the LN latency.
                qT = qkp.tile([128, tok], BF16, tag="qT", name="qT0")
                kT = qkp.tile([128, tok], BF16, tag="kT", name="kT0")
                emit_rstd_col()
                for tt in range(2):
                    emit_v_group(tt)
                emit_qk_slice(0, qT, 0, 0)
                emit_qk_slice(0, kT, 1, 0)
                for tt in range(2, 6):
                    emit_v_group(tt)

                for p in range(NPAIRS):
                    oT = oTp.tile([128, tok], BF16, tag="oT", name=f"oT{p}")
                    oTs.append(oT)
                    nxt = []
                    if p + 1 < NPAIRS:
                        qT2 = qkp.tile([128, tok], BF16, tag="qT",
                                       name=f"qT{p + 1}")
                        kT2 = qkp.tile([128, tok], BF16, tag="kT",
                                       name=f"kT{p + 1}")
                        nxt = [(p + 1, dst, d, nt)
                               for d, dst in ((0, qT2), (1, kT2))
                               for nt in range(NQT)]
                    for t_i in range(NQT):
                        emit_attn_qtile(p, t_i, qT, kT, oT)
                        if p == 0 and t_i < 3:
                            # just-in-time rest of pair 0's QK and V
                            # (V groups 0-5 were emitted up front)
                            emit_qk_slice(0, qT, 0, t_i + 1)
                            emit_qk_slice(0, kT, 1, t_i + 1)
                            for tt in range(4 * t_i + 6, 4 * t_i + 10):
                                if tt < NTT:
                                    emit_v_group(tt)
                        if p == NPAIRS - 1 and t_i < NQT - 1:
                            # out-proj for this qtile's tokens rides along
                            # (the last qtile's blocks run after the pools
                            # close, pairs 0-2 first, so they overlap the
                            # final denominator chain)
                            for tt in range(4 * t_i, 4 * (t_i + 1)):
                                emit_outproj_tt(tt)
                        for _ in range(2):
                            if nxt:
                                emit_qk_slice(*nxt.pop(0))
                    while nxt:
                        emit_qk_slice(*nxt.pop(0))
                    if p + 1 < NPAIRS:
                        qT, kT = qT2, kT2

            # Final out-proj blocks (tokens of the last qtile): pairs 0-2
            # accumulate while pair 3's last denominator chain completes,
            # the pair-3 matmul joins last.
            with tc.tile_pool(name="ps_fin", bufs=6, space="PSUM") as pfin:
                fin = [(tt, nb) for tt in range(4 * (NQT - 1), NTT)
                       for nb in range(DIM // QT)]
                tiles = {}

                def fin_p012(i):
                    tt, nb = fin[i]
                    ps = pfin.tile([128, QT], F32, tag="fin",
                                   name=f"fin{tt}_{nb}")
                    for p in range(NPAIRS - 1):
                        nc.tensor.matmul(
                            out=ps, lhsT=oTs[p][:, tt * KC:(tt + 1) * KC],
                            rhs=wos[p][:, nb * QT:(nb + 1) * QT],
                            start=(p == 0), stop=False)
                    tiles[i] = ps

                for i in range(6):
                    fin_p012(i)
                for i in range(len(fin)):
                    if i >= 6:
                        fin_p012(i)
                    tt, nb = fin[i]
                    nc.tensor.matmul(
                        out=tiles[i],
                        lhsT=oTs[NPAIRS - 1][:, tt * KC:(tt + 1) * KC],
                        rhs=wos[NPAIRS - 1][:, nb * QT:(nb + 1) * QT],
                        start=False, stop=True)
                    ob = outp.tile([128, QT], F32, tag="out_sb")
                    # exp stream is over: split copies over DVE and ACT and
                    # the final 512KB writes over all three DMA queues so
                    # the output drain doesn't serialize on one queue
                    if i % 2 == 0:
                        nc.vector.tensor_copy(ob, tiles.pop(i))
                    else:
                        nc.scalar.copy(ob, tiles.pop(i))
                    [nc.sync, nc.scalar, nc.gpsimd][i % 3].dma_start(
                        out=out[tt * KC:(tt + 1) * KC,
                                nb * QT:(nb + 1) * QT], in_=ob)

    return nc


def make_masks():
    import ml_dtypes

    k = np.arange(KC)[:, None]
    q = np.arange(KC)[None, :]
    return (q >= k).astype(ml_dtypes.bfloat16)


def make_in_maps(x, ln_gamma, ln_beta, w_qkv, w_out):
    import ml_dtypes

    bf16 = ml_dtypes.bfloat16
    x = np.asarray(x, np.float32)
    g_ = np.asarray(ln_gamma, np.float32)
    b_ = np.asarray(ln_beta, np.float32)
    w_qkv = np.asarray(w_qkv, np.float32)
    w_out = np.asarray(w_out, np.float32)
    mask128 = make_masks()
    eye4 = np.eye(4, dtype=np.float32)
    in_maps = []
    for c in range(8):
        b = c // 2
        g = c % 2
        cs = slice(g * INNER_PC, (g + 1) * INNER_PC)
        Wraw = np.concatenate(
            [w_qkv[:, 0 * DIM:1 * DIM][:, cs],
             w_qkv[:, 1 * DIM:2 * DIM][:, cs],
             w_qkv[:, 2 * DIM:3 * DIM][:, cs]], axis=1)
        Wp = (Wraw * g_[:, None]).astype(bf16)
        seed = np.stack([b_ @ Wraw,
                         Wp.astype(np.float32).sum(axis=0)]).astype(bf16)
        in_maps.append({
            "xT": np.ascontiguousarray(x[b].T).astype(bf16),
            "w": np.ascontiguousarray(Wp),
            "seed": seed,
            "wo": np.ascontiguousarray(w_out[cs, :]).astype(bf16),
            "mask128": mask128,
            "eye4": eye4,
        })
    return in_maps


_PROG = None


def kernel(x, ln_gamma, ln_beta, w_qkv, w_out):
    global _PROG
    from concourse.bass_utils import run_bass_kernel_spmd

    if _PROG is None:
        _PROG = build_program(TOK)
    in_maps = make_in_maps(x, ln_gamma, ln_beta, w_qkv, w_out)
    res = run_bass_kernel_spmd(_PROG, in_maps, list(range(8)))
    parts = [res.results[c]["out"] for c in range(8)]
    out = np.empty((B, TOK, DIM), np.float32)
    for b in range(B):
        out[b] = parts[2 * b] + parts[2 * b + 1]
    return out


# revision 51
# speedup vs baseline: 1.1632x; 1.0015x over previous
"""Causal attention block (LN -> QKV -> causal MHA -> out-proj) on 8 trn2
NeuronCores via Bass/Tile.

Sharding: core c handles batch b=c//2 and head-group g=c%2 (8 of 16 heads).
Data parallel over batch, tensor parallel over heads; the out-proj partial
sums (2 per batch) are reduced on the host during the gather, so the device
program needs no collectives and is pure SPMD.

v2 design (bf16 everywhere, PE kept warm, no DRAM roundtrips):
  - all matmul operands bf16 (full-rate 1 cyc/row, FWL weight loads, half
    the DMA); PSUM accumulation stays fp32.
  - LN stats land in a [4, 512] PSUM layout so the postprocessing runs on
    4 partitions instead of 1; rstd is folded into the PSUM->SBUF copies
    (per-partition tensor_scalar for V, a broadcast-row multiply for Q/K),
    so x itself is never rewritten.  The mean/beta correction enters each
    projection as a K=2 seed matmul with rows [std, -mu].
  - scores are computed transposed per 128-key chunk with 2 heads packed;
    exp runs on ACT straight out of PSUM with the 1/sqrt(dh) scale folded
    in; diagonal chunks are narrowed to the valid q-range (saves PE + ACT)
    and only the 128x128 boundary block gets a 0/1 mask multiply.
  - softmax denominators ride as a 65th ones-column of V through the P@V
    matmul; per pair they are DMA-gathered into a [64, 64] tile for one
    batched reciprocal, spread back with two small DMAs, and applied in the
    normalize copy into the SBUF-resident O^T (no DRAM scratch).
  - the QK projection of pair p+1 is interleaved into the (ACT-bound)
    attention stream of pair p so the PE never idles long enough to be
    clock-throttled.
"""

import numpy as np

import concourse.bass as bass
import concourse.mybir as mybir
import concourse.tile as tile_mod

# ----------------------------------------------------------------------------
# Workaround for this walrus build rejecting instructions that carry more than
# MAX_WAITS semaphore waits ("Too many sync wait commands" in CoreV3GenImpl
# setupSyncWait — hit on Drain and Matmult/S3_LW encodings). Split excess
# waits onto single-wait NOP carrier instructions emitted just before the
# original instruction on the same engine: program order on the sequencer
# makes this semantically identical (waits are AND conditions).
# ----------------------------------------------------------------------------
_MAX_WAITS = 1
_orig_add_instruction = tile_mod.TileContext._add_instruction
_carrier_id = [0]


def _split_waits_add_instruction(self, inst):
    si = inst.sync_info
    if (
        si is not None
        and si.on_wait
        and len(si.on_wait) > _MAX_WAITS
        and inst.engine != mybir.EngineType.Unassigned
    ):
        waits = list(si.on_wait)
        keep = waits[-_MAX_WAITS:]
        for w in waits[:-_MAX_WAITS]:
            _carrier_id[0] += 1
            nop = mybir.InstNoOp(name=f"I-waitc-{_carrier_id[0]}")
            nop.engine = inst.engine
            nop.sync_info = mybir.SyncInfo(on_wait=[w], on_update=[])
            _orig_add_instruction(self, nop)
        inst.sync_info = mybir.SyncInfo(
            on_wait=keep,
            on_update=list(si.on_update) if si.on_update else [],
        )
    _orig_add_instruction(self, inst)


tile_mod.TileContext._add_instruction = _split_waits_add_instruction

from concourse.vector_clock import ScopedClock


def _patched_drain_and_barrier(self, tick_clock, wait_clock):
    # Same wait-splitting for the TileContext exit drain, which is emitted
    # after lowering (outside _add_instruction).
    nc = self.nc
    carrier = nc.sync.nop(nofuse=True)
    wait_clock.add_sem_waits(carrier.ins, ScopedClock({None: tick_clock.global_clock}))
    si = carrier.ins.sync_info
    waits = list(si.on_wait) if si is not None and si.on_wait else []
    if len(waits) > _MAX_WAITS:
        carrier.ins.sync_info = mybir.SyncInfo(
            on_wait=waits[:_MAX_WAITS],
            on_update=list(si.on_update) if si.on_update else [],
        )
        rest = waits[_MAX_WAITS:]
        while rest:
            extra = nc.sync.nop(nofuse=True)
            extra.ins.sync_info = mybir.SyncInfo(
                on_wait=rest[:_MAX_WAITS], on_update=[])
            rest = rest[_MAX_WAITS:]

    nc.sync.drain()
    nc.all_engine_barrier()
    assert self.sems is not None
    popped = nc._tile_sem_poison_stack.pop()
    assert popped is self._sem_poison
    nc.clear_and_free_semaphores(list(self.sems.allocated().values()))
    nc.all_engine_barrier()


tile_mod.TileContext._drain_and_barrier = _patched_drain_and_barrier

# ----------------------------------------------------------------------------

F32 = mybir.dt.float32
BF16 = mybir.dt.bfloat16
ALU = mybir.AluOpType
ACT_EXP = mybir.ActivationFunctionType.Exp
ACT_SQRT = mybir.ActivationFunctionType.Sqrt

B = 4
TOK = 2048
DIM = 1024
HEADS = 16
DH = 64
HEADS_PC = 8          # heads per core
INNER_PC = HEADS_PC * DH  # 512
NPAIRS = HEADS_PC // 2
QT = 512              # query tile
KC = 128              # key-token chunk
NQT = TOK // QT       # 4
NTT = TOK // KC       # 16
NKD = DIM // 128      # 8
EPS = 1e-5
SCALE = DH ** -0.5


def _ap0(ap, parts):
    """Partition-broadcast AP (stride-0 leading dim) for DMA reads."""
    return bass.AP(tensor=ap.tensor, offset=ap.offset, ap=[[0, parts]] + list(ap.ap))


def _pstride(t, step, n):
    """View tile t's partitions with a stride (partition p -> p*step)."""
    return bass.AP(tensor=t.tensor, offset=t.offset,
                   ap=[[t.ap[0][0] * step, n]] + list(t.ap[1:]))


def build_program(tok=TOK):
    nc = bass.Bass()
    xT = nc.declare_dram_parameter("xT", [DIM, tok], BF16, isOutput=False)
    w = nc.declare_dram_parameter("w", [DIM, 3 * INNER_PC], BF16, isOutput=False)
    seed = nc.declare_dram_parameter("seed", [2, 3 * INNER_PC], BF16, isOutput=False)
    wo = nc.declare_dram_parameter("wo", [INNER_PC, DIM], BF16, isOutput=False)
    mask_d = nc.declare_dram_parameter("mask128", [KC, KC], BF16, isOutput=False)
    eye_d = nc.declare_dram_parameter("eye4", [4, 4], F32, isOutput=False)
    out = nc.declare_dram_parameter("out", [tok, DIM], F32, isOutput=True)
    # DRAM bounce buffers for partition-broadcasts (stride-0 reads are only
    # legal on the DRAM side of a DMA)
    rstd_d = nc.dram_tensor("rstd_row", [1, tok], F32)
    rec_d = nc.dram_tensor("rec_row", [NPAIRS, 8 * QT], BF16)

    with tile_mod.TileContext(nc) as tc, nc.allow_low_precision(
            "bf16 operand tiles; all matmul accumulation stays fp32 PSUM"):
        with (
            tc.tile_pool(name="const", bufs=1) as const,
            tc.tile_pool(name="xt", bufs=NKD) as xtp,
            tc.tile_pool(name="wsb", bufs=NKD) as wp,
            tc.tile_pool(name="wosb", bufs=NPAIRS) as wop,
            tc.tile_pool(name="vpool", bufs=NTT) as vpool,
            tc.tile_pool(name="qkT", bufs=2) as qkp,
            tc.tile_pool(name="osb", bufs=3) as osbp,
            tc.tile_pool(name="oT", bufs=NPAIRS) as oTp,
            tc.tile_pool(name="psb", bufs=3) as ppool,
            tc.tile_pool(name="den", bufs=2) as denp,
            tc.tile_pool(name="sq", bufs=2) as sqp,
            tc.tile_pool(name="out_sb", bufs=6) as outp,
        ):
            # ---------------- constants / big loads ----------------
            ones_col = const.tile([128, 1], BF16, tag="ones_col")
            nc.vector.memset(ones_col, 1.0)
            # sel4: 4 stationary variants [128, 4]; variant v is all-ones in
            # column v, zero elsewhere.  Routes LN stat rows to consecutive
            # PSUM partitions (matmul/DVE PSUM bases must be 32-aligned).
            sel4 = const.tile([128, 16], BF16, tag="sel4")
            nc.vector.memset(sel4, 0.0)
            for v in range(4):
                nc.vector.memset(sel4[:, 5 * v:5 * v + 1], 1.0)
            eps4 = const.tile([4, 1], F32, tag="eps")
            nc.vector.memset(eps4, EPS)
            mask_sb = const.tile([KC, KC], BF16, tag="mask")
            nc.sync.dma_start(out=mask_sb, in_=mask_d[:, :])
            eye4 = const.tile([4, 4], F32, tag="eye4")
            nc.sync.dma_start(out=eye4, in_=eye_d[:, :])
            seed_sb = const.tile([2, 3 * INNER_PC], BF16, tag="seed")
            nc.sync.dma_start(out=seed_sb, in_=seed[:, :])

            # x chunks arrive split in token-halves across four engine
            # queues so the transfers run in parallel and the LN stats start
            # consuming as early as possible.
            xt = []
            qeng = [nc.sync, nc.scalar, nc.gpsimd]
            for kc in range(NKD):
                t = xtp.tile([128, tok], BF16, tag="xt")
                for hf in range(2):
                    hsl = slice(hf * (tok // 2), (hf + 1) * (tok // 2))
                    qeng[(2 * kc + hf) % 3].dma_start(
                        out=t[:, hsl],
                        in_=xT[kc * 128:(kc + 1) * 128, hsl])
                xt.append(t)
            wsb = []
            for kc in range(NKD):
                t = wp.tile([128, 3 * INNER_PC], BF16, tag="wsb")
                nc.sync.dma_start(out=t, in_=w[kc * 128:(kc + 1) * 128, :])
                wsb.append(t)
            wos = []
            for p in range(NPAIRS):
                t = wop.tile([128, DIM], BF16, tag="wosb")
                nc.sync.dma_start(out=t, in_=wo[p * 128:(p + 1) * 128, :])
                wos.append(t)

            # LN-derived rows (device computed, long-lived)
            onm = const.tile([2, tok], BF16, tag="onm")        # r0=std r1=-mu
            bc_sb = const.tile([128, tok], F32, tag="bc")      # rstd bcast
            # NOTE: rstd_col columns are block-permuted: chunk tt lives at
            # column 4*(tt%4) + tt//4 (transpose blocks land contiguously).
            rstd_col = const.tile([128, NTT], F32, tag="rstdc")
            rstd4 = const.tile([4, QT], F32, tag="rstd4")
            std4 = const.tile([4, QT], F32, tag="std4")
            std4b = const.tile([4, QT], BF16, tag="std4b")
            nmu4b = const.tile([4, QT], BF16, tag="nmu4b")
            mu4 = const.tile([4, QT], F32, tag="mu4")
            var4 = const.tile([4, QT], F32, tag="var4")
            musq4 = const.tile([4, QT], F32, tag="musq4")

            # ---------------- phase A: LN stats ----------------
            with (
                tc.tile_pool(name="ps_stats", bufs=1, space="PSUM") as pstat,
            ):
                # Warm-up: dummy ap-512 matmuls on memset constants (no
                # DMA dependency) sized to span the whole input-DMA wait
                # (~10us), so the HAM clock gate is open (2.4 GHz) when the
                # stats matmuls start and stays open through them.
                warm_rhs = const.tile([128, QT], BF16, tag="warm_rhs")
                nc.vector.memset(warm_rhs, 1.0)
                warm_ps = pstat.tile([4, QT], F32, tag="warm")
                for _ in range(30):
                    nc.tensor.matmul(out=warm_ps, lhsT=sel4[:, 0:4],
                                     rhs=warm_rhs, start=True, stop=True)
                # Two [4, 512] PSUM tiles: token-slice nt's sum / sum-of-sq
                # rows land on partition nt via the sel4 stationary (other
                # rows accumulate zero), one accumulation group per tile.
                sum_ps = pstat.tile([4, QT], F32, tag="sum")
                sq_ps = pstat.tile([4, QT], F32, tag="sq")
                for kc in range(NKD):
                    for nt in range(NQT):
                        sl = slice(nt * QT, (nt + 1) * QT)
                        first = kc == 0 and nt == 0
                        last = kc == NKD - 1 and nt == NQT - 1
                        sq_t = sqp.tile([128, QT], BF16, tag="sq_t")
                        nc.vector.tensor_mul(sq_t, xt[kc][:, sl], xt[kc][:, sl])
                        nc.tensor.matmul(
                            out=sum_ps, lhsT=sel4[:, 4 * nt:4 * nt + 4],
                            rhs=xt[kc][:, sl],
                            start=first, stop=last)
                        nc.tensor.matmul(
                            out=sq_ps, lhsT=sel4[:, 4 * nt:4 * nt + 4],
                            rhs=sq_t,
                            start=first, stop=last)
                # postproc on [4, 512] (4 lanes)
                nc.vector.tensor_scalar_mul(mu4, sum_ps, 1.0 / DIM)
                nc.vector.tensor_scalar_mul(var4, sq_ps, 1.0 / DIM)
                nc.vector.tensor_mul(musq4, mu4, mu4)
                nc.vector.tensor_sub(var4, var4, musq4)
                nc.scalar.activation(out=std4, in_=var4, func=ACT_SQRT,
                                     bias=eps4, scale=1.0)
                nc.vector.reciprocal(rstd4, std4)
                nc.vector.tensor_copy(std4b, std4)
                nc.vector.tensor_scalar_mul(nmu4b, mu4, -1.0)
                # gather LN rows into operand layouts (cross-shape DMAs:
                # only total element count must match)
                # rstd broadcast [128, tok]: bounce through DRAM, then
                # stride-0 partition-broadcast reads (split per 512-slice so
                # the first Q/K copies unblock as early as possible)
                nc.gpsimd.dma_start(out=rstd_d[0:1, :], in_=rstd4[:, :])
                nc.gpsimd.dma_start(out=onm[0:1, :], in_=std4b[:, :])
                nc.gpsimd.dma_start(out=onm[1:2, :], in_=nmu4b[:, :])
                for j in range(NQT):
                    nc.gpsimd.dma_start(
                        out=bc_sb[:, j * QT:(j + 1) * QT],
                        in_=bass.AP(tensor=rstd_d, offset=j * QT,
                                    ap=[[0, 128], [1, QT]]))
            # ------- phases B-D: projections + attention + out-proj -------
            # One shared [128, 512]-f32 PSUM pool ("proj") serves the QK
            # slices, the V groups, the rstd transposes and the out-proj
            # groups; they never overlap in time.  8 banks total:
            # proj 2 + scores 4 + o_ps 2.
            v_sb = [None] * NTT
            oTs = []
            with (
                tc.tile_pool(name="ps_proj", bufs=2, space="PSUM") as pproj,
                tc.tile_pool(name="ps_s", bufs=2, space="PSUM") as pss,
                tc.tile_pool(name="ps_o", bufs=2, space="PSUM") as pso,
            ):
                def emit_qk_slice(p, dst, d, nt):
                    """One [128, 512] token-slice of the Q or K projection of
                    pair p (d=0 -> Q, d=1 -> K).  Seed matmul last so the
                    group never waits on the LN postprocessing."""
                    sl = slice(nt * QT, (nt + 1) * QT)
                    cofs = d * INNER_PC + p * 128
                    ps = pproj.tile([128, QT], F32, tag="proj", name="qk_ps")
                    for kc in range(NKD):
                        nc.tensor.matmul(
                            out=ps, lhsT=wsb[kc][:, cofs:cofs + 128],
                            rhs=xt[kc][:, sl],
                            start=(kc == 0), stop=False)
                    nc.tensor.matmul(
                        out=ps, lhsT=seed_sb[:, cofs:cofs + 128],
                        rhs=onm[:, sl], start=False, stop=True)
                    nc.vector.tensor_mul(dst[:, sl], ps, bc_sb[:, sl])

                def emit_rstd_col():
                    # 4 PE transposes of [4, 128] blocks; block g lands at
                    # contiguous cols [4g, 4g+4) (permuted order, see above).
                    rc_ps = pproj.tile([128, QT], F32, tag="proj", name="rc")
                    for g in range(4):
                        nc.tensor.transpose(
                            out=rc_ps[:, 4 * g:4 * (g + 1)],
                            in_=rstd4[:, g * 128:(g + 1) * 128],
                            identity=eye4)
                    nc.vector.tensor_copy(rstd_col, rc_ps[:, 0:NTT])

                def emit_v_group(tt):
                    tsl = slice(tt * KC, (tt + 1) * KC)
                    v_ps = pproj.tile([128, INNER_PC], F32, tag="proj",
                                      name="v_ps")
                    for kc in range(NKD):
                        nc.tensor.matmul(
                            out=v_ps, lhsT=xt[kc][:, tsl],
                            rhs=wsb[kc][:, 2 * INNER_PC:3 * INNER_PC],
                            start=(kc == 0), stop=False)
                    nc.tensor.matmul(
                        out=v_ps, lhsT=onm[:, tsl],
                        rhs=seed_sb[:, 2 * INNER_PC:3 * INNER_PC],
                        start=False, stop=True)
                    vt = vpool.tile([128, HEADS_PC * (DH + 1)], BF16,
                                    tag="v_sb", name=f"v_sb{tt}")
                    v3 = vt.rearrange("p (h w) -> p h w", w=DH + 1)
                    pc = 4 * (tt % 4) + tt // 4  # permuted rstd_col index
                    nc.vector.tensor_scalar(
                        out=v3[:, :, 0:DH],
                        in0=v_ps.rearrange("p (h w) -> p h w", w=DH),
                        scalar1=rstd_col[:, pc:pc + 1], scalar2=None,
                        op0=ALU.mult)
                    nc.vector.memset(v3[:, :, DH:DH + 1], 1.0)
                    v_sb[tt] = vt

                def emit_outproj_tt(tt):
                    tsl = slice(tt * KC, (tt + 1) * KC)
                    for nb in range(DIM // QT):
                        nsl = slice(nb * QT, (nb + 1) * QT)
                        ps = pproj.tile([128, QT], F32, tag="proj",
                                        name="out_ps")
                        for p in range(NPAIRS):
                            nc.tensor.matmul(
                                out=ps, lhsT=oTs[p][:, tsl],
                                rhs=wos[p][:, nsl],
                                start=(p == 0), stop=(p == NPAIRS - 1))
                        ob = outp.tile([128, QT], F32, tag="out_sb")
                        nc.vector.tensor_copy(ob, ps)
                        nc.sync.dma_start(out=out[tsl, nsl], in_=ob)

                def emit_attn_qtile(p, t_i, qT, kT, oT):
                    """Scores/exp/mask/PV for one query tile, then the
                    per-qtile denominator chain and normalize into oT."""
                    qsl0 = t_i * QT
                    nch = (t_i + 1) * QT // KC
                    o_ps = [pso.tile([DH + 1, QT], F32, tag="o_ps",
                                     name=f"o_ps{p}_{t_i}_{h}")
                            for h in range(2)]
                    p_tiles = {}

                    def emit_scores(c):
                        m = c - (nch - 4)
                        lo = 128 * m if m > 0 else 0
                        csl = slice(c * KC, (c + 1) * KC)
                        s_ps = pss.tile([128, 2 * QT], F32, tag="s_ps")
                        p_sb = ppool.tile([128, 2 * QT], BF16, tag="p_sb")
                        for h in range(2):
                            nc.tensor.matmul(
                                out=s_ps[:, h * QT + lo:(h + 1) * QT],
                                lhsT=kT[h * DH:(h + 1) * DH, csl],
                                rhs=qT[h * DH:(h + 1) * DH,
                                       qsl0 + lo:qsl0 + QT],
                                start=True, stop=True)
                        s3 = s_ps.rearrange("p (h q) -> p h q", q=QT)
                        p3 = p_sb.rearrange("p (h q) -> p h q", q=QT)
                        nc.scalar.activation(
                            out=p3[:, :, lo:QT], in_=s3[:, :, lo:QT],
                            func=ACT_EXP, scale=SCALE)
                        if m >= 0:
                            for h in range(2):
                                nc.vector.tensor_mul(
                                    p_sb[:, h * QT + lo:h * QT + lo + KC],
                                    p_sb[:, h * QT + lo:h * QT + lo + KC],
                                    mask_sb)
                        p_tiles[c] = p_sb

                    def emit_pv(c):
                        m = c - (nch - 4)
                        lo = 128 * m if m > 0 else 0
                        p_sb = p_tiles.pop(c)
                        for h in range(2):
                            hc = (2 * p + h) * (DH + 1)
                            nc.tensor.matmul(
                                out=o_ps[h][:, lo:QT],
                                lhsT=v_sb[c][:, hc:hc + DH + 1],
                                rhs=p_sb[:, h * QT + lo:(h + 1) * QT],
                                start=(c == 0), stop=(c == nch - 1),
                                skip_group_check=True)

                    emit_scores(0)
                    for c in range(1, nch):
                        emit_scores(c)
                        emit_pv(c - 1)
                    emit_pv(nch - 1)
                    # free PSUM fast (O^T rows + denominator row 64)
                    o_sb = [osbp.tile([DH + 1, QT], BF16, tag=f"o_sb{h}",
                                      name=f"o_sb{p}_{t_i}_{h}")
                            for h in range(2)]
                    for h in range(2):
                        nc.vector.tensor_copy(o_sb[h], o_ps[h])
                    # per-qtile denominator chain: den16 partition 8h+j
                    # holds tokens [64j, 64j+64) of head h; flattened
                    # partition-major this gives rec_d offsets 512h+64j+e.
                    den16 = denp.tile([16, 64], BF16, tag="den16")
                    rec16 = denp.tile([16, 64], BF16, tag="rec16")
                    rb_q = denp.tile([64, 2 * QT], BF16, tag="rb_q")
                    for h in range(2):
                        nc.gpsimd.dma_start(
                            out=den16[8 * h:8 * h + 8, :],
                            in_=o_sb[h][DH:DH + 1, :])
                    nc.vector.reciprocal(rec16, den16)
                    dofs = p * 8 * QT + t_i * 2 * QT
                    nc.gpsimd.dma_start(
                        out=bass.AP(tensor=rec_d, offset=dofs,
                                    ap=[[2 * QT, 1], [1, 2 * QT]]),
                        in_=rec16[:, :])
                    nc.gpsimd.dma_start(
                        out=rb_q,
                        in_=bass.AP(tensor=rec_d, offset=dofs,
                                    ap=[[0, 64], [1, 2 * QT]]))
                    qsl = slice(qsl0, qsl0 + QT)
                    for h in range(2):
                        nc.vector.tensor_mul(
                            oT[h * DH:(h + 1) * DH, qsl],
                            o_sb[h][0:DH, :],
                            rb_q[:, h * QT:(h + 1) * QT])

                # ---- schedule ----
                # V groups lead: their PSUM-freeing copies gate only on
                # rstd_col (ready right after the postproc), while the q/k
                # copies wait for the slower bc broadcast chain — so V in
                # front keeps the proj pool cycling during the LN latency.
                qT = qkp.tile([128, tok], BF16, tag="qT", name="qT0")
                kT = qkp.tile([128, tok], BF16, tag="kT", name="kT0")
                emit_rstd_col()
                for tt in range(2):
                    emit_v_group(tt)
                emit_qk_slice(0, qT, 0, 0)
                emit_qk_slice(0, kT, 1, 0)
                for tt in range(2, 6):
                    emit_v_group(tt)

                for p in range(NPAIRS):
                    oT = oTp.tile([128, tok], BF16, tag="oT", name=f"oT{p}")
                    oTs.append(oT)
                    nxt = []
                    if p + 1 < NPAIRS:
                        qT2 = qkp.tile([128, tok], BF16, tag="qT",
                                       name=f"qT{p + 1}")
                        kT2 = qkp.tile([128, tok], BF16, tag="kT",
                                       name=f"kT{p + 1}")
                        nxt = [(p + 1, dst, d, nt)
                               for d, dst in ((0, qT2), (1, kT2))
                               for nt in range(NQT)]
                    for t_i in range(NQT):
                        emit_attn_qtile(p, t_i, qT, kT, oT)
                        if p == 0 and t_i < 3:
                            # just-in-time rest of pair 0's QK and V
                            # (V groups 0-5 were emitted up front)
                            emit_qk_slice(0, qT, 0, t_i + 1)
                            emit_qk_slice(0, kT, 1, t_i + 1)
                            for tt in range(4 * t_i + 6, 4 * t_i + 10):
                                if tt < NTT:
                                    emit_v_group(tt)
                        if p == NPAIRS - 1 and t_i < NQT - 1:
                            # out-proj for this qtile's tokens rides along
                            # (the last qtile's blocks run after the pools
                            # close, pairs 0-2 first, so they overlap the
                            # final denominator chain)
                            for tt in range(4 * t_i, 4 * (t_i + 1)):
                                emit_outproj_tt(tt)
                        for _ in range(2):
                            if nxt:
                                emit_qk_slice(*nxt.pop(0))
                    while nxt:
                        emit_qk_slice(*nxt.pop(0))
                    if p + 1 < NPAIRS:
                        qT, kT = qT2, kT2

            # Final out-proj blocks (tokens of the last qtile): pairs 0-2
            # accumulate while pair 3's last denominator chain completes,
            # the pair-3 matmul joins last.
            with tc.tile_pool(name="ps_fin", bufs=6, space="PSUM") as pfin:
                fin = [(tt, nb) for tt in range(4 * (NQT - 1), NTT)
                       for nb in range(DIM // QT)]
                tiles = {}

                def fin_p012(i):
                    tt, nb = fin[i]
                    ps = pfin.tile([128, QT], F32, tag="fin",
                                   name=f"fin{tt}_{nb}")
                    for p in range(NPAIRS - 1):
                        nc.tensor.matmul(
                            out=ps, lhsT=oTs[p][:, tt * KC:(tt + 1) * KC],
                            rhs=wos[p][:, nb * QT:(nb + 1) * QT],
                            start=(p == 0), stop=False)
                    tiles[i] = ps

                for i in range(6):
                    fin_p012(i)
                for i in range(len(fin)):
                    if i >= 6:
                        fin_p012(i)
                    tt, nb = fin[i]
                    nc.tensor.matmul(
                        out=tiles[i],
                        lhsT=oTs[NPAIRS - 1][:, tt * KC:(tt + 1) * KC],
                        rhs=wos[NPAIRS - 1][:, nb * QT:(nb + 1) * QT],
                        start=False, stop=True)
                    ob = outp.tile([128, QT], F32, tag="out_sb")
                    # exp stream is over: split copies over DVE and ACT and
                    # the final 512KB writes over all three DMA queues so
                    # the output drain doesn't serialize on one queue
                    if i % 2 == 0:
                        nc.vector.tensor_copy(ob, tiles.pop(i))
                    else:
                        nc.scalar.copy(ob, tiles.pop(i))
                    [nc.sync, nc.scalar, nc.gpsimd][i % 3].dma_start(
                        out=out[tt * KC:(tt + 1) * KC,
                                nb * QT:(nb + 1) * QT], in_=ob)

    return nc


def make_masks():
    import ml_dtypes

    k = np.arange(KC)[:, None]
    q = np.arange(KC)[None, :]
    return (q >= k).astype(ml_dtypes.bfloat16)


def make_in_maps(x, ln_gamma, ln_beta, w_qkv, w_out):
    import ml_dtypes

    bf16 = ml_dtypes.bfloat16
    x = np.asarray(x, np.float32)
    g_ = np.asarray(ln_gamma, np.float32)
    b_ = np.asarray(ln_beta, np.float32)
    w_qkv = np.asarray(w_qkv, np.float32)
    w_out = np.asarray(w_out, np.float32)
    mask128 = make_masks()
    eye4 = np.eye(4, dtype=np.float32)
    in_maps = []
    for c in range(8):
        b = c // 2
        g = c % 2
        cs = slice(g * INNER_PC, (g + 1) * INNER_PC)
        Wraw = np.concatenate(
            [w_qkv[:, 0 * DIM:1 * DIM][:, cs],
             w_qkv[:, 1 * DIM:2 * DIM][:, cs],
             w_qkv[:, 2 * DIM:3 * DIM][:, cs]], axis=1)
        Wp = (Wraw * g_[:, None]).astype(bf16)
        seed = np.stack([b_ @ Wraw,
                         Wp.astype(np.float32).sum(axis=0)]).astype(bf16)
        in_maps.append({
            "xT": np.ascontiguousarray(x[b].T).astype(bf16),
            "w": np.ascontiguousarray(Wp),
            "seed": seed,
            "wo": np.ascontiguousarray(w_out[cs, :]).astype(bf16),
            "mask128": mask128,
            "eye4": eye4,
        })
    return in_maps


_PROG = None


def kernel(x, ln_gamma, ln_beta, w_qkv, w_out):
    global _PROG
    from concourse.bass_utils import run_bass_kernel_spmd

    if _PROG is None:
        _PROG = build_program(TOK)
    in_maps = make_in_maps(x, ln_gamma, ln_beta, w_qkv, w_out)
    res = run_bass_kernel_spmd(_PROG, in_maps, list(range(8)))
    parts = [res.results[c]["out"] for c in range(8)]
    out = np.empty((B, TOK, DIM), np.float32)
    for b in range(B):
        out[b] = parts[2 * b] + parts[2 * b + 1]
    return out
